# revision 16
# baseline (speedup 1.0000x reference)
"""AttentionBlock (GroupNorm + single-head self-attention + proj + residual)
for Trainium2, 8 NeuronCores, data-parallel over (batch, token-half).

Shapes (hardcoded): x [4, 256, 64, 64] fp32, weights [256, 256] fp32.
Each core handles one (batch b, token-half h): 2048 query tokens against the
full 4096 keys/values of its batch, entirely in SBUF.

Primary kernel: build_nc_fp8 (fp8e4m3 DoubleRow, channel-major attention,
measured ~136us HW, rel err ~5.4e-3). Legacy fp16 kernel kept as build_nc
(~186-205us, rel err ~1.2e-4) and used as the runtime fallback.

Structure per core (build_nc_fp8):
  - The host passes x[b] with the core's query half rotated to columns
    0..2047 (attention is token-order invariant), so the program is SPMD
    with no dynamic offsets.
  - GroupNorm is folded into the QKV weights (of = seff*x + beff): K/Q/V
    come from raw x with runtime-scaled weights; beff maps to a dropped
    per-query softmax constant (K), a Q bias, and a V-shift folded into the
    output bias. GN statistics come from one bn_stats pass per 512-chunk.
  - All large matmuls are fp8e4m3 with perf_mode=DoubleRow (256-deep
    contraction per pass, 2x the fp16 PE rate): QKV production, scores
    S^T = K^T Q (keys on partitions), and PV with V as the stationary so
    the attention output lands channel-major [c, q] — consumed directly by
    the projection, eliminating the fp16 kernel's 32 PE transposes.
  - fp8 range handling: q,k,v scaled by 8 (folded into seff), u = exp/16
    (folded into the exp bias), keeping every operand within e4m3's +-240.
  - Softmax denominator: an all-8.0 fp8 DoubleRow matmul per key pair-tile
    accumulates column sums of u in PSUM (broadcast across partitions for
    free); DVE reciprocal + multiply normalizes during the PSUM drain.
    Value algebra: acc_ps = acc/2, den_ps = den/2 => acc_ps/den_ps exact.
  - The residual rides the projection's PSUM accumulation as an f32r
    identity-matmul; ACT adds the output bias during the PSUM->SBUF drain.
  - exp splits across engines per pair-tile: ACT exact exp (5/7) and a
    DVE 2-op Schraudolph (tensor_scalar affine + relu-to-int8, whose bits
    ARE the fp8 value) for the rest; the attention loop is software-
    pipelined (scores/exp lead PV/den by 2 tiles) so the in-order PE never
    head-of-line blocks on an exp.
  - HW-validated constraints: gpsimd must not touch PSUM; one PSUM operand
    per DVE op; f32r matmul inputs must be f32r-typed; multi-bank PSUM
    reads on ACT are catastrophically slow (208us vs 136us!); custom DVE
    ops (InstCustomDveAnt) do not compile on the pinned walrus.
"""

import sys

try:
    import concourse.bass as bass  # noqa: F401
except ImportError:
    sys.path.insert(0, "/opt/trn_rl_repo")

import numpy as np

import concourse.bass as bass
import concourse.mybir as mybir
import concourse.tile as tile
from concourse.bass import ts
from concourse.bass_utils import run_bass_kernel_spmd
from concourse.masks import make_identity

FP = mybir.dt.float32
FPR = mybir.dt.float32r
AF = mybir.ActivationFunctionType
ALU = mybir.AluOpType
AX = mybir.AxisListType

P = 128
C = 256
HW = 4096
HALF = 2048
NCH = 2          # channel chunks of 128
NJT = 32         # 128-wide key tiles
NIG = 4          # query i-groups of 512
NCHUNK = 8       # 512-wide token chunks of the full image
GROUPS = 32
GSIZE = C // GROUPS          # 8 channels per group
NELEM = GSIZE * HW           # 32768 elements per group
EPS = 1e-6
SCALE = float(C) ** -0.5     # 0.0625


def _split_waits(nc, max_waits=1):
    """The pinned walrus rejects >1 sync-wait on ctrl instructions; hoist
    excess waits onto preceding NoOps on the same engine (same instruction
    stream, so ordering is preserved)."""
    ctr = 0
    for bb in nc.m.functions[0].blocks:
        out = []
        changed = False
        for inst in bb.instructions:
            si = getattr(inst, "sync_info", None)
            waits = list(si.on_wait) if (si and si.on_wait) else []
            if len(waits) > max_waits:
                changed = True
                head, rest = waits[:-max_waits], waits[-max_waits:]
                for k in range(0, len(head), max_waits):
                    ctr += 1
                    nop = mybir.InstNoOp(name=f"I-wsplit-{ctr}", ins=[], outs=[])
                    nop.engine = inst.engine
                    nop.sync_info = mybir.SyncInfo(
                        on_wait=head[k : k + max_waits], on_update=[]
                    )
                    out.append(nop)
                inst.sync_info = mybir.SyncInfo(
                    on_wait=rest, on_update=list(si.on_update or [])
                )
            out.append(inst)
        if changed:
            bb.instructions = out


def build_nc(split=True, reps=1, f32r=True, pv16=True):
    MD = FPR if f32r else FP          # dtype for matmul-feeding SBUF tiles
    # Attention operands (k, q, u = exp(scores), V) tolerate fp16: 5e-4
    # element rounding perturbs softmax scores by ~5e-4 absolute and averages
    # out over the 4096-term sums (~1e-4 end-to-end), and fp16 stationary
    # operands get the fast (overlapped) LDWEIGHTS path that 4-byte f32r
    # lacks. Production matmuls and the projection stay f32r/fp32.
    MH = mybir.dt.float16 if (f32r and pv16) else (FPR if f32r else FP)

    def mdcast(ap):
        return ap.bitcast(FPR) if f32r else ap

    nc = bass.Bass()
    xf = nc.dram_tensor("xf", [C, HW], FP, kind="ExternalInput")
    wqT = nc.dram_tensor("wqT", [C, C], FP, kind="ExternalInput")
    wkT = nc.dram_tensor("wkT", [C, C], FP, kind="ExternalInput")
    wvT = nc.dram_tensor("wvT", [C, C], FP, kind="ExternalInput")
    wpT = nc.dram_tensor("wpT", [C, C], FP, kind="ExternalInput")
    bq2 = nc.dram_tensor("bq2", [P, NCH], FP, kind="ExternalInput")
    bp2 = nc.dram_tensor("bp2", [P, NCH], FP, kind="ExternalInput")
    gns = nc.dram_tensor("gns", [P, NCH], FP, kind="ExternalInput")
    gnb = nc.dram_tensor("gnb", [P, NCH], FP, kind="ExternalInput")
    gsel = nc.dram_tensor("gsel", [P, P], FP, kind="ExternalInput")
    y = nc.dram_tensor("y", [C, HALF], FP, kind="ExternalOutput")

    with tile.TileContext(nc) as tc:
        with (
            tc.tile_pool(name="wts", bufs=1) as wts,
            tc.tile_pool(name="big", bufs=1) as big,
            tc.tile_pool(name="upool", bufs=8) as upool,
            tc.tile_pool(name="small", bufs=3) as small,
            tc.tile_pool(name="stats", bufs=1) as stats,
            tc.tile_pool(name="outp", bufs=3) as outp,
            tc.tile_pool(name="psA", bufs=4, space="PSUM") as psA,
            tc.tile_pool(name="psAcc", bufs=4, space="PSUM") as psAcc,
        ):
            # ---------------- input image first (critical path), then constants
            xf_sb = big.tile([P, NCH, HW], MD, tag="xf")
            # spread the input-image chunks across engine DMA queues so they
            # land in parallel — the groupnorm stats (and so every matmul)
            # serialize behind the last chunk
            dma_engines = [nc.sync, nc.gpsimd, nc.scalar]
            for o in range(NCH):
                for t8 in range(NCHUNK):
                    eng = dma_engines[(o * NCHUNK + t8) % len(dma_engines)]
                    eng.dma_start(
                        out=xf_sb[:, o, ts(t8, 512)],
                        in_=mdcast(xf[o * P : (o + 1) * P, ts(t8, 512)]),
                    )

            # ---------------- constants + input image ----------------
            w_sb = {}
            for name, dram in (("wqT", wqT), ("wkT", wkT), ("wvT", wvT), ("wpT", wpT)):
                t = wts.tile([P, NCH, C], MD, tag=f"w_{name}")
                nc.sync.dma_start(
                    out=t, in_=mdcast(dram.rearrange("(o p) c -> p o c", p=P))
                )
                w_sb[name] = t
            gsel_sb = wts.tile([P, P], FP, tag="gsel")
            nc.sync.dma_start(out=gsel_sb, in_=gsel[:, :])
            bq_sb = wts.tile([P, NCH], FP, tag="bq")
            nc.sync.dma_start(out=bq_sb, in_=bq2[:, :])
            bp_sb = wts.tile([P, NCH], FP, tag="bp")
            nc.sync.dma_start(out=bp_sb, in_=bp2[:, :])
            gns_sb = wts.tile([P, NCH], FP, tag="gns")
            nc.sync.dma_start(out=gns_sb, in_=gns[:, :])
            gnb_sb = wts.tile([P, NCH], FP, tag="gnb")
            nc.sync.dma_start(out=gnb_sb, in_=gnb[:, :])
            ident_fp = wts.tile([P, P], FP, tag="ident_fp")
            make_identity(nc, ident_fp)
            ident = wts.tile([P, P], MD, tag="ident")
            nc.vector.tensor_copy(ident, ident_fp)

            def xfp(o, sl):  # fp32 view of resident x for exact stats/residual
                return xf_sb[:, o, sl].bitcast(FP) if f32r else xf_sb[:, o, sl]

            # fp16 copy of x for the QKV production matmuls (overlapped LDW)
            if f32r and pv16:
                x16 = big.tile([P, NCH, HW], MH, tag="x16")
                for o in range(NCH):
                    for t8 in range(NCHUNK):
                        nc.vector.tensor_copy(
                            x16[:, o, ts(t8, 512)], xfp(o, ts(t8, 512))
                        )
            else:
                x16 = xf_sb

            for _rep in range(reps):
                # ---------------- phase 1a: GroupNorm statistics ----------------
                sum_cols = stats.tile([P, NCH, NCHUNK], FP, tag="sumc")
                sq_cols = stats.tile([P, NCH, NCHUNK], FP, tag="sqc")
                for t8 in range(NCHUNK):
                    for o in range(NCH):
                        nc.vector.tensor_reduce(
                            out=sum_cols[:, o, t8 : t8 + 1],
                            in_=xfp(o, ts(t8, 512)),
                            axis=AX.X, op=ALU.add,
                        )
                        sq_scr = small.tile([P, 512], FP, tag="sqscr")
                        nc.scalar.activation(
                            out=sq_scr, in_=xfp(o, ts(t8, 512)), func=AF.Square,
                            accum_out=sq_cols[:, o, t8 : t8 + 1],
                        )

                seff = stats.tile([P, NCH], FP, tag="seff")
                beff = stats.tile([P, NCH], FP, tag="beff")
                eps_sb = stats.tile([P, 1], FP, tag="eps")
                nc.vector.memset(eps_sb, EPS)
                for o in range(NCH):
                    part = stats.tile([P, 2], FP, tag=f"part{o}")
                    nc.vector.tensor_reduce(
                        out=part[:, 0:1], in_=sum_cols[:, o], axis=AX.X, op=ALU.add
                    )
                    nc.vector.tensor_reduce(
                        out=part[:, 1:2], in_=sq_cols[:, o], axis=AX.X, op=ALU.add
                    )
                    gps = psA.tile([P, 512], FP, tag="ps512", name="gps")[:, :2]
                    nc.tensor.matmul(gps, lhsT=gsel_sb, rhs=part, start=True, stop=True)
                    mean = stats.tile([P, 1], FP, tag=f"mean{o}")
                    nc.vector.tensor_scalar_mul(mean, gps[:, 0:1], 1.0 / NELEM)
                    ex2 = stats.tile([P, 1], FP, tag=f"ex2{o}")
                    nc.vector.tensor_scalar_mul(ex2, gps[:, 1:2], 1.0 / NELEM)
                    msq = stats.tile([P, 1], FP, tag=f"msq{o}")
                    nc.vector.tensor_mul(msq, mean, mean)
                    var = stats.tile([P, 1], FP, tag=f"var{o}")
                    nc.vector.tensor_tensor(var, ex2, msq, ALU.subtract)
                    # rstd = exp(-0.5 * ln(var + eps)) — stays in the exp table set
                    lnv = stats.tile([P, 1], FP, tag=f"lnv{o}")
                    nc.scalar.activation(out=lnv, in_=var, func=AF.Ln, bias=eps_sb)
                    rstd = stats.tile([P, 1], FP, tag=f"rstd{o}")
                    nc.scalar.activation(out=rstd, in_=lnv, func=AF.Exp, scale=-0.5)
                    nc.vector.tensor_mul(seff[:, o : o + 1], gns_sb[:, o : o + 1], rstd)
                    tmp = stats.tile([P, 1], FP, tag=f"tmp{o}")
                    nc.vector.tensor_mul(tmp, mean, seff[:, o : o + 1])
                    nc.vector.tensor_tensor(
                        beff[:, o : o + 1], gnb_sb[:, o : o + 1], tmp, ALU.subtract
                    )

                # ---------------- phase 1b: fold GN into the weights ----------------
                # bias matvecs on the raw weights first: qb = wq@beff + bq,
                # vb = wv@beff, pvb = wp@vb; then scale wq/wk/wv by seff in place
                def matvec(wname, rhs_sb, out_tile):
                    # plain fp32 matmuls (N=1 is not f32r-legal; cost is trivial)
                    for oo in range(NCH):
                        mv = psA.tile([P, 512], FP, tag="ps512", name="mv")[:, :1]
                        for oi in range(NCH):
                            lhs = w_sb[wname][:, oi, oo * P : (oo + 1) * P]
                            nc.tensor.matmul(
                                mv,
                                lhsT=lhs.bitcast(FP) if f32r else lhs,
                                rhs=rhs_sb[:, oi : oi + 1],
                                start=(oi == 0), stop=(oi == NCH - 1),
                            )
                        nc.vector.tensor_copy(out_tile[:, oo : oo + 1], mv)

                qb = stats.tile([P, NCH], FP, tag="qb")
                vb = stats.tile([P, NCH], FP, tag="vb")
                pvb = stats.tile([P, NCH], FP, tag="pvb")
                ob = stats.tile([P, NCH], FP, tag="ob")
                matvec("wqT", beff, qb)
                nc.vector.tensor_add(qb, qb, bq_sb)
                matvec("wvT", beff, vb)
                matvec("wpT", vb, pvb)
                nc.vector.tensor_add(ob, pvb, bp_sb)

                w2 = {}
                for wname in ("wqT", "wkT", "wvT"):
                    w2[wname] = wts.tile([P, NCH, C], MH, tag=f"w2_{wname}", name=f"w2{wname}")
                    for o in range(NCH):
                        nc.vector.tensor_scalar_mul(
                            w2[wname][:, o], w_sb[wname][:, o], seff[:, o : o + 1]
                        )

                # ---------------- phase 1c: K, V'^T, Q ----------------
                k_sb = big.tile([P, NCH, HW], MH, tag="k")
                v_sb = big.tile([P, NJT, C + 2], MH, tag="v")
                if MH == FPR:
                    nc.vector.memset(v_sb[:, :, C : C + 1].bitcast(FP), 1.0)
                    nc.vector.memset(v_sb[:, :, C + 1 : C + 2].bitcast(FP), 0.0)
                else:
                    nc.vector.memset(v_sb[:, :, C : C + 1], 1.0)
                    nc.vector.memset(v_sb[:, :, C + 1 : C + 2], 0.0)
                for t8 in range(NCHUNK):
                    for oo in range(NCH):
                        ps = psA.tile([P, 512], FP, tag="ps512", name="psk")
                        for oi in range(NCH):
                            nc.tensor.matmul(
                                ps,
                                lhsT=w2["wkT"][:, oi, oo * P : (oo + 1) * P],
                                rhs=x16[:, oi, ts(t8, 512)],
                                start=(oi == 0), stop=(oi == NCH - 1),
                            )
                        if (t8 + oo) % 2 == 0:
                            nc.vector.tensor_copy(
                                out=k_sb[:, oo, ts(t8, 512)], in_=ps
                            )
                        else:
                            nc.scalar.copy(out=k_sb[:, oo, ts(t8, 512)], in_=ps)
                    for jj in range(4):
                        j = t8 * 4 + jj
                        ps = psA.tile([P, 512], FP, tag="ps512", name="psv")[:, :C]
                        for oi in range(NCH):
                            nc.tensor.matmul(
                                ps,
                                lhsT=x16[:, oi, j * P : (j + 1) * P],
                                rhs=w2["wvT"][:, oi],
                                start=(oi == 0), stop=(oi == NCH - 1),
                            )
                        if j % 2 == 0:
                            nc.vector.tensor_copy(out=v_sb[:, j, 0:C], in_=ps)
                        else:
                            nc.scalar.copy(out=v_sb[:, j, 0:C], in_=ps)

                q_sb = big.tile([P, NCH, HALF], MH, tag="q")
                for oo in range(NCH):
                    for i4 in range(4):
                        ps = psA.tile([P, 512], FP, tag="ps512", name="psq")
                        for oi in range(NCH):
                            nc.tensor.matmul(
                                ps,
                                lhsT=w2["wqT"][:, oi, oo * P : (oo + 1) * P],
                                rhs=x16[:, oi, ts(i4, 512)],
                                start=(oi == 0), stop=(oi == NCH - 1),
                            )
                        nc.vector.tensor_scalar_add(
                            q_sb[:, oo, ts(i4, 512)], ps, qb[:, oo : oo + 1]
                        )

                # ---------------- phase 2: attention ----------------
                att_sb = big.tile([P, NCH, HALF], MD, tag="att")
                for g in range(NIG):
                    acc = [
                        psAcc.tile([P, C + 2], FP, tag="acc", name=f"acc{g}_{t}")
                        for t in range(4)
                    ]
                    for j in range(NJT):
                        ps = psA.tile([P, 512], FP, tag="ps512", name="pss")
                        for o in range(NCH):
                            nc.tensor.matmul(
                                ps,
                                lhsT=k_sb[:, o, j * P : (j + 1) * P],
                                rhs=q_sb[:, o, ts(g, 512)],
                                start=(o == 0), stop=(o == NCH - 1),
                            )
                        u = upool.tile([P, 512], MH, tag="u")
                        nc.scalar.activation(out=u, in_=ps, func=AF.Exp, scale=SCALE)
                        for t in range(4):
                            nc.tensor.matmul(
                                acc[t],
                                lhsT=u[:, t * P : (t + 1) * P],
                                rhs=v_sb[:, j],
                                start=(j == 0), stop=(j == NJT - 1),
                            )
                    for t in range(4):
                        rs = small.tile([P, 1], FP, tag="rs")
                        nc.vector.reciprocal(rs, acc[t][:, C : C + 1])
                        asb = small.tile([P, C], MD, tag="asb")
                        nc.vector.tensor_scalar_mul(asb, acc[t][:, 0:C], rs)
                        for o in range(NCH):
                            tps = psAcc.tile([P, P], FP, tag="acc", name="tps")
                            nc.tensor.transpose(
                                tps.bitcast(FPR) if f32r else tps,
                                asb[:, o * P : (o + 1) * P],
                                ident,
                            )
                            col = g * 512 + t * P
                            nc.vector.tensor_copy(
                                out=att_sb[:, o, col : col + P], in_=tps
                            )
                    # projection + residual; the LAST group uses quarter
                    # chunks so its ACT/DVE/DMA drain pipelines instead of
                    # serializing behind PE's final matmul
                    sub = 2 if g == NIG - 1 else 1
                    w_sub = 512 // sub
                    for oo in range(NCH):
                        for s in range(sub):
                            col = g * 512 + s * w_sub
                            ps = psA.tile([P, 512], FP, tag="ps512", name="psp")[
                                :, :w_sub
                            ]
                            for oi in range(NCH):
                                nc.tensor.matmul(
                                    ps,
                                    lhsT=w_sb["wpT"][:, oi, oo * P : (oo + 1) * P],
                                    rhs=att_sb[:, oi, col : col + w_sub],
                                    start=(oi == 0), stop=(oi == NCH - 1),
                                )
                            ot = outp.tile([P, 512], FP, tag="out", name="ot")[
                                :, :w_sub
                            ]
                            nc.scalar.activation(
                                out=ot, in_=ps, func=AF.Identity,
                                bias=ob[:, oo : oo + 1],
                            )
                            nc.vector.tensor_add(
                                ot, ot, xfp(oo, slice(col, col + w_sub))
                            )
                            nc.sync.dma_start(
                                out=y[oo * P : (oo + 1) * P, col : col + w_sub],
                                in_=ot,
                            )

    if split:
        _split_waits(nc)
    return nc


def build_nc_fp8(split=True, reps=1, prod8=True, exp_dve=(2, (0,)),
                 den_pairs=False, fused_dve=True, pipe_stats=True):
    """fp8-DoubleRow rewrite: channel-major attention, no PE transposes.

    - All big matmuls run fp8e4m3 with perf_mode=DoubleRow (contraction 256
      per pass, 2 muls/cell/cycle): QKV production, scores S^T = K^T Q
      (keys on partitions), PV with V as the stationary operand so the
      attention output lands channel-major [c, q], and the projection
      (att in fp8) — no PE transposes anywhere.
    - Score scaling is chosen so the PSUM score IS the Schraudolph
      exponent-bits affine: q8 = SQK*(q+qb), k8 = SQK*k with
      SQK^2 = 8*log2e*C^-0.5, so sps = 8*log2e*s_true and the DVE exp is a
      single fused op: i8 = max(sps + B, 0) with int8 convert, whose bits
      ARE the fp8 value of u = exp(s_true)/32. ACT tiles compute the exact
      exp via scale=1/(8 log2e), bias=-ln32.
    - v8 = 8*v, u = exp/32, ones value 8.0: acc_ps = acc/4, den_ps = den/4,
      so acc_ps * recip(den_ps) = acc/den exactly.
    - Softmax denominator rides the PE as ones-matmuls, but 7 of every 8
      key pair-tiles per i-group are first pair-summed on the otherwise-idle
      gpsimd (u/32 keeps pair sums within e4m3's 240), halving most den
      passes: 9 den passes per i-group instead of 16.
    """
    F8 = mybir.dt.float8e4
    F16 = mybir.dt.float16
    DRM = mybir.MatmulPerfMode.DoubleRow
    NJP = NJT // 2  # 16 key pair-tiles
    LOG2E = 1.4426950408889634
    SQK = (8.0 * LOG2E * (float(C) ** -0.5)) ** 0.5  # 0.84932... (q and k each)
    ACT_SCALE = 1.0 / (8.0 * LOG2E)  # sps -> s_true for the ACT exact exp
    MLN32 = -3.4657359027997265  # -ln(32)
    # i8 = sps + B with u = exp(s_true - ln32): 8*(log2 u + 7) = sps + 16;
    # -0.32 centers the mantissa-interpolation error
    EXP8_B = 16.0 - 0.32

    nc = bass.Bass()
    xf = nc.dram_tensor("xf", [C, HW], FP, kind="ExternalInput")
    wqT = nc.dram_tensor("wqT", [C, C], FP, kind="ExternalInput")
    wkT = nc.dram_tensor("wkT", [C, C], FP, kind="ExternalInput")
    wvT = nc.dram_tensor("wvT", [C, C], FP, kind="ExternalInput")
    wpT = nc.dram_tensor("wpT", [C, C], FP, kind="ExternalInput")
    bq2 = nc.dram_tensor("bq2", [P, NCH], FP, kind="ExternalInput")
    bp2 = nc.dram_tensor("bp2", [P, NCH], FP, kind="ExternalInput")
    gns = nc.dram_tensor("gns", [P, NCH], FP, kind="ExternalInput")
    gnb = nc.dram_tensor("gnb", [P, NCH], FP, kind="ExternalInput")
    gsel = nc.dram_tensor("gsel", [P, P], FP, kind="ExternalInput")
    y = nc.dram_tensor("y", [C, HALF], FP, kind="ExternalOutput")

    with tile.TileContext(nc) as tc:
        with (
            tc.tile_pool(name="wts", bufs=1) as wts,
            tc.tile_pool(name="big", bufs=1) as big,
            tc.tile_pool(name="upool", bufs=8) as upool,
            tc.tile_pool(name="uspool", bufs=2) as uspool,
            tc.tile_pool(name="attp", bufs=2) as attp,
            tc.tile_pool(name="small", bufs=3) as small,
            tc.tile_pool(name="stats", bufs=1) as stats,
            tc.tile_pool(name="outp", bufs=3) as outp,
            tc.tile_pool(name="psS", bufs=2, space="PSUM") as psS,      # 2x2 banks
            tc.tile_pool(name="psAcc", bufs=2, space="PSUM") as psAcc,  # 2 banks
            tc.tile_pool(name="psD", bufs=1, space="PSUM") as psD,      # 1 bank
            tc.tile_pool(name="psP", bufs=1, space="PSUM") as psP,      # 1 bank
        ):
            # ---------------- input image first (critical path), then constants
            # stored as f32r so the residual identity-matmul may read it
            # natively; fp32 consumers use xfp() bitcast views
            xf_r = big.tile([P, NCH, HW], FPR, tag="xf")
            dma_engines = [nc.sync, nc.gpsimd, nc.scalar]
            for o in range(NCH):
                for t8 in range(NCHUNK):
                    eng = dma_engines[(o * NCHUNK + t8) % len(dma_engines)]
                    eng.dma_start(
                        out=xf_r[:, o, ts(t8, 512)],
                        in_=xf[o * P : (o + 1) * P, ts(t8, 512)].bitcast(FPR),
                    )
            xf_sb = xf_r.bitcast(FP)

            w_sb = {}
            for name, dram in (("wqT", wqT), ("wkT", wkT), ("wvT", wvT), ("wpT", wpT)):
                t = wts.tile([P, NCH, C], FP, tag=f"w_{name}")
                nc.sync.dma_start(
                    out=t, in_=dram.rearrange("(o p) c -> p o c", p=P)
                )
                w_sb[name] = t
            gsel_sb = wts.tile([P, P], FP, tag="gsel")
            nc.sync.dma_start(out=gsel_sb, in_=gsel[:, :])
            bq_sb = wts.tile([P, NCH], FP, tag="bq")
            nc.sync.dma_start(out=bq_sb, in_=bq2[:, :])
            bp_sb = wts.tile([P, NCH], FP, tag="bp")
            nc.sync.dma_start(out=bp_sb, in_=bp2[:, :])
            gns_sb = wts.tile([P, NCH], FP, tag="gns")
            nc.sync.dma_start(out=gns_sb, in_=gns[:, :])
            gnb_sb = wts.tile([P, NCH], FP, tag="gnb")
            nc.sync.dma_start(out=gnb_sb, in_=gnb[:, :])

            # fp8 all-0.5 stationary for the PE-side softmax-denominator
            # accumulation: u8 = exp/32 => den_ps = den/64, and with
            # acc_ps = acc/4 (v8 = 8v) the normalize acc_ps*recip(den_ps)
            # yields 16*att — the fp8 att scale the projection expects
            ones8 = wts.tile([P, 2, P], F8, tag="ones8")
            nc.vector.memset(ones8, 0.5)
            # residual identity is pre-scaled x256 to compensate the fp8
            # projection's operand scaling (wp8 = 16*wp, att8 = 16*att); the
            # ACT drain divides the whole PSUM by 256 before adding ob
            ident_fp = wts.tile([P, P], FP, tag="ident_fp")
            make_identity(nc, ident_fp)
            ident_fp256 = wts.tile([P, P], FP, tag="ident_fp256")
            nc.vector.tensor_scalar_mul(ident_fp256, ident_fp, 256.0)
            ident = wts.tile([P, P], FPR, tag="ident")
            nc.vector.tensor_copy(ident, ident_fp256)
            mln32 = wts.tile([P, 1], FP, tag="mln32")
            nc.vector.memset(mln32, MLN32)

            # fp8 projection weights, x16 so ~N(0,1/256) entries sit in
            # e4m3's normal range; att8 is x16 likewise (via ones8 = 0.5)
            wp8 = wts.tile([P, NCH, C], F8, tag="wp8")
            for o in range(NCH):
                nc.vector.tensor_scalar_mul(wp8[:, o], w_sb["wpT"][:, o], 16.0)

            # fp8 copy of x for the QKV production matmuls (ACT + Pool so the
            # DVE stays free for the bn_stats groupnorm pass)
            x8 = big.tile([P, NCH, HW], F8, tag="x8")
            for o in range(NCH):
                for t8 in range(NCHUNK):
                    m = (o * NCHUNK + t8) % 4
                    if m in (0, 2):
                        nc.scalar.copy(
                            out=x8[:, o, ts(t8, 512)], in_=xf_sb[:, o, ts(t8, 512)]
                        )
                    elif m == 1:
                        nc.gpsimd.tensor_copy(
                            x8[:, o, ts(t8, 512)], xf_sb[:, o, ts(t8, 512)]
                        )
                    else:
                        nc.vector.tensor_copy(
                            x8[:, o, ts(t8, 512)], xf_sb[:, o, ts(t8, 512)]
                        )

            for _rep in range(reps):
                # ---------------- phase 1a: GroupNorm statistics ----------------
                # chunks split across engines so the serial head shrinks:
                # DVE bn_stats ((count, mean, M2) for even/odd elements) for
                # most chunks, ACT Square/Identity + accum_out (direct
                # sum/sumsq; both funcs share the exp table set) for the rest
                NACT = 5
                NDVE = NCHUNK - NACT
                bnt = stats.tile([P, NCH, NDVE, 6], FP, tag="bnt")
                sumc = stats.tile([P, NCH, NACT], FP, tag="sumc")
                sqc = stats.tile([P, NCH, NACT], FP, tag="sqc")
                for t8 in range(NCHUNK):
                    for o in range(NCH):
                        if t8 < NDVE:
                            nc.vector.bn_stats(
                                out=bnt[:, o, t8, :], in_=xf_sb[:, o, ts(t8, 512)]
                            )
                        else:
                            ia = t8 - NDVE
                            scr = small.tile([P, 512], FP, tag="scr")
                            nc.scalar.activation(
                                out=scr, in_=xf_sb[:, o, ts(t8, 512)],
                                func=AF.Square, accum_out=sqc[:, o, ia : ia + 1],
                            )
                            scr2 = small.tile([P, 512], FP, tag="scr")
                            nc.scalar.activation(
                                out=scr2, in_=xf_sb[:, o, ts(t8, 512)],
                                func=AF.Identity,
                                accum_out=sumc[:, o, ia : ia + 1],
                            )

                seff = stats.tile([P, NCH], FP, tag="seff")
                beff = stats.tile([P, NCH], FP, tag="beff")
                eps_sb = stats.tile([P, 1], FP, tag="eps")
                nc.vector.memset(eps_sb, EPS)
                for o in range(NCH):
                    me = bnt[:, o, :, 1]
                    mo = bnt[:, o, :, 4]
                    m2e = bnt[:, o, :, 2]
                    m2o = bnt[:, o, :, 5]
                    tm = stats.tile([P, NDVE], FP, tag=f"tm{o}")
                    nc.vector.tensor_tensor(tm, me, mo, ALU.add)
                    t2 = stats.tile([P, NDVE], FP, tag=f"t2{o}")
                    nc.vector.tensor_tensor(t2, m2e, m2o, ALU.add)
                    sqm = stats.tile([P, 2 * NDVE], FP, tag=f"sqm{o}")
                    nc.vector.tensor_tensor(sqm[:, 0:NDVE], me, me, ALU.mult)
                    nc.vector.tensor_tensor(sqm[:, NDVE:], mo, mo, ALU.mult)
                    red = stats.tile([P, 5], FP, tag=f"red{o}")
                    nc.vector.tensor_reduce(
                        out=red[:, 0:1], in_=tm, axis=AX.X, op=ALU.add
                    )
                    nc.vector.tensor_reduce(
                        out=red[:, 1:2], in_=t2, axis=AX.X, op=ALU.add
                    )
                    nc.vector.tensor_reduce(
                        out=red[:, 2:3], in_=sqm, axis=AX.X, op=ALU.add
                    )
                    nc.vector.tensor_reduce(
                        out=red[:, 3:4], in_=sumc[:, o], axis=AX.X, op=ALU.add
                    )
                    nc.vector.tensor_reduce(
                        out=red[:, 4:5], in_=sqc[:, o], axis=AX.X, op=ALU.add
                    )
                    part = stats.tile([P, 2], FP, tag=f"part{o}")
                    nc.vector.scalar_tensor_tensor(
                        part[:, 0:1], red[:, 0:1], 256.0, red[:, 3:4],
                        ALU.mult, ALU.add,
                    )
                    p1t = stats.tile([P, 1], FP, tag=f"p1t{o}")
                    nc.vector.scalar_tensor_tensor(
                        p1t, red[:, 2:3], 256.0, red[:, 1:2],
                        ALU.mult, ALU.add,
                    )
                    nc.vector.tensor_tensor(
                        part[:, 1:2], p1t, red[:, 4:5], ALU.add
                    )
                    gps = psD.tile([P, 512], FP, tag="psd", name="gps")[:, :2]
                    nc.tensor.matmul(gps, lhsT=gsel_sb, rhs=part, start=True, stop=True)
                    mean = stats.tile([P, 1], FP, tag=f"mean{o}")
                    nc.vector.tensor_scalar_mul(mean, gps[:, 0:1], 1.0 / NELEM)
                    ex2 = stats.tile([P, 1], FP, tag=f"ex2{o}")
                    nc.vector.tensor_scalar_mul(ex2, gps[:, 1:2], 1.0 / NELEM)
                    msq = stats.tile([P, 1], FP, tag=f"msq{o}")
                    nc.vector.tensor_mul(msq, mean, mean)
                    var = stats.tile([P, 1], FP, tag=f"var{o}")
                    nc.vector.tensor_tensor(var, ex2, msq, ALU.subtract)
                    lnv = stats.tile([P, 1], FP, tag=f"lnv{o}")
                    nc.scalar.activation(out=lnv, in_=var, func=AF.Ln, bias=eps_sb)
                    rstd = stats.tile([P, 1], FP, tag=f"rstd{o}")
                    nc.scalar.activation(out=rstd, in_=lnv, func=AF.Exp, scale=-0.5)
                    nc.vector.tensor_mul(seff[:, o : o + 1], gns_sb[:, o : o + 1], rstd)
                    tmp = stats.tile([P, 1], FP, tag=f"tmp{o}")
                    nc.vector.tensor_mul(tmp, mean, seff[:, o : o + 1])
                    nc.vector.tensor_tensor(
                        beff[:, o : o + 1], gnb_sb[:, o : o + 1], tmp, ALU.subtract
                    )

                # ---------------- phase 1b: fold GN into the weights ----------------
                def matvec(wname, rhs_sb, out_tile):
                    for oo in range(NCH):
                        mv = psD.tile([P, 512], FP, tag="psd", name="mv")[:, :1]
                        for oi in range(NCH):
                            lhs = w_sb[wname][:, oi, oo * P : (oo + 1) * P]
                            nc.tensor.matmul(
                                mv, lhsT=lhs, rhs=rhs_sb[:, oi : oi + 1],
                                start=(oi == 0), stop=(oi == NCH - 1),
                            )
                        nc.vector.tensor_copy(out_tile[:, oo : oo + 1], mv)

                qb = stats.tile([P, NCH], FP, tag="qb")
                vb = stats.tile([P, NCH], FP, tag="vb")
                pvb = stats.tile([P, NCH], FP, tag="pvb")
                ob = stats.tile([P, NCH], FP, tag="ob")
                matvec("wqT", beff, qb)
                nc.vector.tensor_add(qb, qb, bq_sb)
                matvec("wvT", beff, vb)
                matvec("wpT", vb, pvb)
                nc.vector.tensor_add(ob, pvb, bp_sb)
                qb8 = stats.tile([P, NCH], FP, tag="qb8")
                nc.vector.tensor_scalar_mul(qb8, qb, SQK)
                seffqk = stats.tile([P, NCH], FP, tag="seffqk")
                nc.vector.tensor_scalar_mul(seffqk, seff, SQK)
                seffv = stats.tile([P, NCH], FP, tag="seffv")
                nc.vector.tensor_scalar_mul(seffv, seff, 8.0)

                w8 = {}
                for wname in ("wqT", "wkT", "wvT"):
                    sc_tile = seffv if wname == "wvT" else seffqk
                    w8[wname] = wts.tile([P, NCH, C], F8, tag=f"w8_{wname}", name=f"w8{wname}")
                    for o in range(NCH):
                        nc.gpsimd.tensor_scalar_mul(
                            w8[wname][:, o], w_sb[wname][:, o], sc_tile[:, o : o + 1]
                        )

                # ---------------- phase 1c: K, V (token-major), Q — fp8 DR ----------
                k8 = big.tile([P, NCH, HW], F8, tag="k8")
                for t8 in range(NCHUNK):
                    for oo in range(NCH):
                        ps = psS.tile([P, 2, 512], FP, tag="pss", name="psk")
                        nc.tensor.matmul(
                            ps[:, 0, :],
                            lhsT=w8["wkT"][:, :, oo * P : (oo + 1) * P],
                            rhs=x8[:, :, ts(t8, 512)],
                            start=True, stop=True, perf_mode=DRM,
                        )
                        if (t8 + oo) % 2 == 0:
                            nc.scalar.copy(
                                out=k8[:, oo, ts(t8, 512)], in_=ps[:, 0, :]
                            )
                        else:
                            nc.vector.tensor_copy(
                                k8[:, oo, ts(t8, 512)], ps[:, 0, :]
                            )


                q8 = big.tile([P, NCH, HALF], F8, tag="q8")
                for i4 in range(4):
                    for oo in range(NCH):
                        ps = psAcc.tile([P, 512], FP, tag="psa", name="psq")
                        nc.tensor.matmul(
                            ps,
                            lhsT=w8["wqT"][:, :, oo * P : (oo + 1) * P],
                            rhs=x8[:, :, ts(i4, 512)],
                            start=True, stop=True, perf_mode=DRM,
                        )
                        if (i4 + oo) % 2 == 0:
                            nc.vector.tensor_scalar_add(
                                q8[:, oo, ts(i4, 512)], ps, qb8[:, oo : oo + 1]
                            )
                        else:
                            nc.scalar.activation(
                                out=q8[:, oo, ts(i4, 512)], in_=ps,
                                func=AF.Identity, bias=qb8[:, oo : oo + 1],
                            )

                v8 = big.tile([P, NJT, C], F8, tag="v8")

                def v_prod_pair(jp):
                    # two key-tiles' V into one PSUM bank, one cast out
                    # (gpsimd cannot read PSUM, so casts live on ACT/DVE)
                    vps = psP.tile([P, 2, 256], FP, tag="psp", name="psv")
                    for h in range(2):
                        nc.tensor.matmul(
                            vps[:, h, :],
                            lhsT=x8[:, :, (2 * jp + h) * P : (2 * jp + h + 1) * P],
                            rhs=w8["wvT"],
                            start=True, stop=True, perf_mode=DRM,
                        )
                    dst = v8[:, 2 * jp : 2 * jp + 2, :]
                    if jp % 2 == 0:
                        nc.scalar.copy(out=dst, in_=vps)
                    else:
                        nc.vector.tensor_copy(dst, vps)

                # ---------------- phase 2: attention (channel-major) ----------------
                # Software-pipelined: scores(jp+1)+exp(jp+1) are emitted BEFORE
                # PV(jp) so the in-order PE never head-of-line blocks on the
                # exp of the tile it is about to consume. exp alternates
                # ACT/DVE (Schraudolph) so consecutive exps overlap.
                # V-production matmuls ride inside g=0 (PV(jp) only needs
                # v8[2jp:2jp+2]), removing them from the serial head.
                def scores_exp(g, jp):
                    sps = psS.tile([P, 2, 512], FP, tag="pss", name="sps")
                    u8t = upool.tile([P, 2, 512], F8, tag="u8")
                    idx = g * NJP + jp
                    on_dve = idx % exp_dve[0] in exp_dve[1]
                    for h in range(2):
                        j = 2 * jp + h
                        nc.tensor.matmul(
                            sps[:, h, :],
                            lhsT=k8[:, :, j * P : (j + 1) * P],
                            rhs=q8[:, :, ts(g, 512)],
                            start=True, stop=True, perf_mode=DRM,
                        )
                    for h in range(2):
                        if on_dve and fused_dve:
                            # stock-op Schraudolph into fp8 bits, fully
                            # fused: the score scaling already makes the
                            # PSUM value the exponent-bits affine, so one
                            # 2-op tensor_scalar (add intercept, relu) with
                            # int8 output convert produces the fp8 bytes
                            nc.vector.tensor_scalar(
                                out=u8t[:, h, :].bitcast(mybir.dt.int8),
                                in0=sps[:, h, :],
                                scalar1=EXP8_B, scalar2=0.0,
                                op0=ALU.add, op1=ALU.max,
                            )
                        elif on_dve:
                            scr = small.tile([P, 512], FP, tag="scr", name="scr")
                            nc.vector.tensor_scalar(
                                out=scr, in0=sps[:, h, :],
                                scalar1=1.0, scalar2=EXP8_B,
                                op0=ALU.mult, op1=ALU.add,
                            )
                            nc.vector.tensor_scalar_max(
                                u8t[:, h, :].bitcast(mybir.dt.int8), scr, 0.0
                            )
                        else:
                            nc.scalar.activation(
                                out=u8t[:, h, :], in_=sps[:, h, :],
                                func=AF.Exp, scale=ACT_SCALE, bias=mln32,
                            )
                    return u8t

                acc = {}
                dps = {}

                def pv(g, jp, u8t):
                    for oc in range(NCH):
                        nc.tensor.matmul(
                            acc[g][oc],
                            lhsT=v8[:, 2 * jp : 2 * jp + 2, oc * P : (oc + 1) * P],
                            rhs=u8t,
                            start=(jp == 0), stop=(jp == NJP - 1),
                            perf_mode=DRM,
                        )

                def den(g, jp, u8s, t2):
                    # softmax denominator on the PE: ones stationary sums u8
                    # over keys (and broadcasts to all 128 partitions). For
                    # jp<=13, pairs of key tiles are pre-summed on gpsimd
                    # (fp8-safe: u/32 keeps pair sums under 240), so one DR
                    # pass covers 512 keys; the last two tiles go direct so
                    # the epilogue never waits on a gpsimd add.
                    if den_pairs and jp % 2 == 1 and jp <= 13:
                        us = uspool.tile(
                            [P, 2, 512], F8, tag="us", name=f"us{g}_{jp // 2}"
                        )
                        nc.gpsimd.tensor_tensor(
                            us, u8s.pop(t2 - 1), u8s.pop(t2), ALU.add
                        )
                        nc.tensor.matmul(
                            dps[g], lhsT=ones8, rhs=us,
                            start=(jp == 1), stop=False, perf_mode=DRM,
                        )
                    elif (not den_pairs) or jp >= 14:
                        nc.tensor.matmul(
                            dps[g], lhsT=ones8, rhs=u8s.pop(t2),
                            start=(jp == 0 and not den_pairs),
                            stop=(jp == NJP - 1), perf_mode=DRM,
                        )

                def epilogue(g):
                    att_g = attp.tile([P, NCH, 512], F8, tag="att", name=f"att{g}")
                    # normalize: DVE reciprocal (PSUM->SBUF) then per-half
                    # multiply (one PSUM operand per DVE op); lands 16*att
                    # in fp8 for the DoubleRow projection
                    recip = small.tile([P, 512], FP, tag="recip", name=f"rc{g}")
                    nc.vector.reciprocal(recip, dps[g])
                    for oc in range(NCH):
                        nc.vector.tensor_tensor(
                            att_g[:, oc, :], acc[g][oc], recip, ALU.mult
                        )
                    # fp8 DR projection (one pass per oo); the residual rides
                    # the accumulation group as a 256-scaled f32r identity-
                    # matmul; ACT drains with scale 1/256 then adds ob
                    for oo in range(NCH):
                        pps = psP.tile([P, 512], FP, tag="psp", name=f"pp{g}_{oo}")
                        nc.tensor.matmul(
                            pps,
                            lhsT=wp8[:, :, oo * P : (oo + 1) * P],
                            rhs=att_g,
                            start=True, stop=False, perf_mode=DRM,
                        )
                        nc.tensor.matmul(
                            pps,
                            lhsT=ident,
                            rhs=xf_r[:, oo, ts(g, 512)],
                            start=False, stop=True,
                        )
                        ot = outp.tile([P, 512], FP, tag="out", name=f"ot{g}_{oo}")
                        nc.scalar.activation(
                            out=ot, in_=pps, func=AF.Identity,
                            scale=1.0 / 256.0, bias=ob[:, oo : oo + 1],
                        )
                        oeng = dma_engines[(g * NCH + oo) % len(dma_engines)]
                        oeng.dma_start(
                            out=y[oo * P : (oo + 1) * P, ts(g, 512)], in_=ot
                        )

                # flat software pipeline over (g, jp): PV/den lag the
                # scores/exp stream by LOOKAHEAD tiles, epilogues follow
                # their last PV, so the in-order PE always has independent
                # work queued ahead of any instruction that waits on
                # ACT/DVE/gpsimd results.
                LOOKAHEAD = 3
                T = NIG * NJP
                u8s = {}
                for t in range(T + LOOKAHEAD):
                    if t < T:
                        g, jp = divmod(t, NJP)
                        if g == 0:
                            if jp == 0:
                                v_prod_pair(0)
                            if jp + 1 < NJP:
                                v_prod_pair(jp + 1)
                        if jp == 0:
                            acc[g] = [
                                psAcc.tile(
                                    [P, 512], FP, tag="psa", name=f"acc{g}_{oc}"
                                )
                                for oc in range(NCH)
                            ]
                            dps[g] = psD.tile([P, 512], FP, tag="psd", name=f"dps{g}")
                        u8s[t] = scores_exp(g, jp)
                    if t >= LOOKAHEAD:
                        t2 = t - LOOKAHEAD
                        g2, jp2 = divmod(t2, NJP)
                        pv(g2, jp2, u8s[t2])
                        den(g2, jp2, u8s, t2)
                        if jp2 == NJP - 1:
                            epilogue(g2)

    if split:
        _split_waits(nc)
    return nc


BUILD = build_nc_fp8

_NC_CACHE = None


def _get_nc():
    global _NC_CACHE
    if _NC_CACHE is None:
        _NC_CACHE = build_nc_fp8()
    return _NC_CACHE


def make_in_maps(x, gn_scale, gn_bias, wq, bq, wk, bk, wv, bv, wp, bp):
    B = x.shape[0]
    f32 = np.float32

    def col2(v):  # [256] -> [128, 2], column o = channels o*128..o*128+127
        return np.ascontiguousarray(np.asarray(v, f32).reshape(NCH, P).T)

    wqT = np.ascontiguousarray(np.asarray(wq, f32).T)
    wkT = np.ascontiguousarray(np.asarray(wk, f32).T)
    wvT = np.ascontiguousarray(np.asarray(wv, f32).T)
    wpT = np.ascontiguousarray(np.asarray(wp, f32).T)
    bp2 = col2(np.asarray(wp, f32) @ np.asarray(bv, f32) + np.asarray(bp, f32))
    bq2 = col2(bq)
    gns = col2(gn_scale)
    gnb = col2(gn_bias)
    gsel = np.kron(np.eye(P // GSIZE, dtype=f32), np.ones((GSIZE, GSIZE), f32))

    xr = np.asarray(x, f32).reshape(B, C, HW)
    in_maps = []
    for core in range(8):
        b, h = core // 2, core % 2
        # rotate so this core's query half sits at columns 0..HALF-1
        # (attention/groupnorm are token-order invariant)
        xfb = np.ascontiguousarray(np.roll(xr[b], -h * HALF, axis=1))
        in_maps.append(
            {
                "xf": xfb,
                "wqT": wqT, "wkT": wkT, "wvT": wvT, "wpT": wpT,
                "bq2": bq2, "bp2": bp2, "gns": gns, "gnb": gnb, "gsel": gsel,
            }
        )
    return in_maps


def assemble_out(results, B=4):
    out = np.empty((B, C, HW), np.float32)
    for core in range(8):
        b, h = core // 2, core % 2
        out[b, :, h * HALF : (h + 1) * HALF] = results[core]["y"]
    return out.reshape(B, C, 64, 64)


def kernel(**inputs):
    in_maps = make_in_maps(**inputs)
    try:
        nc = _get_nc()
        res = run_bass_kernel_spmd(nc, in_maps, list(range(8)))
    except Exception:
        # fallback: exact-fp32 variant (4x slower matmuls) after letting a
        # possibly-wedged device recover
        import time as _time

        _time.sleep(90)
        nc = build_nc(f32r=False)
        res = run_bass_kernel_spmd(nc, in_maps, list(range(8)))
    return assemble_out(res.results, B=inputs["x"].shape[0])


if __name__ == "__main__":
    rng = np.random.default_rng(0)
    ins = {
        "x": rng.standard_normal((4, C, 64, 64)).astype(np.float32),
        "gn_scale": np.ones(C, np.float32),
        "gn_bias": np.zeros(C, np.float32),
    }
    for w in ("wq", "wk", "wv", "wp"):
        ins[w] = (rng.standard_normal((C, C)) / 16.0).astype(np.float32)
    for b in ("bq", "bk", "bv", "bp"):
        ins[b] = np.zeros(C, np.float32)
    out = kernel(**ins)
    print(out.shape, out.dtype, float(np.abs(out).mean()))



# revision 24
# speedup vs baseline: 1.0547x; 1.0547x over previous
"""AttentionBlock (GroupNorm + single-head self-attention + proj + residual)
for Trainium2, 8 NeuronCores, data-parallel over (batch, token-half).

Shapes (hardcoded): x [4, 256, 64, 64] fp32, weights [256, 256] fp32.
Each core handles one (batch b, token-half h): 2048 query tokens against the
full 4096 keys/values of its batch, entirely in SBUF.

v3 "Z-scheme" kernel (build_nc_fp8): the scores bilinear form is
S = of^T (wk^T wq) of, so the host precomputes M = wk^T wq (weight-only
preprocessing) and the kernel never materializes K or Q at all:

  - Z[g] = M8T^T @ x8[queries of i-group g]   (2 fp8-DR passes per g)
    with M8T = fp8(16 * seff ⊙ M^T) folding the query-side GroupNorm
    scale; the drain z8 = A2 ⊙ zps + B2 folds the key-side seff, the
    query bias (w2 = wk^T bq, host-precomputed) and M @ beff.
  - S^T tile = x8_keytile^T @ z8  (keys on partitions, fp8 DR),
    identical cost to the old K^T Q but with NO K/Q production matmuls
    and NO k8/q8 PSUM drains (-16 PE passes, -40 vector-engine drains).
  - Score scaling LAM = 8*log2e*C^-0.5 makes the PSUM score the
    Schraudolph exponent-bits affine: the DVE exp is ONE fused op
    i8 = max(sps + B, 0) with int8 convert whose bits ARE fp8(exp/32);
    ACT tiles use the exact exp (scale 1/(8 log2e), bias -ln32).
  - u = exp/32, v8 = 8v, ones = 0.5: acc_ps = acc/4, den_ps = den/64,
    so acc_ps/den_ps = 16*att — the fp8 scale the DR projection expects
    with wp8 = fp8(16 wp); the 256x is removed in the ACT drain (the
    residual identity-matmul is pre-scaled by 256).
  - GroupNorm stats: 1024-wide chunks, split DVE bn_stats / ACT
    Square+Identity accum (all funcs share one ACT table set); with
    reps>1 the next rep's chunk stats are emitted DURING this rep's
    attention (x is rep-invariant), removing them from the serial head.
  - HW-validated constraints respected: gpsimd never touches PSUM; one
    PSUM operand per DVE op; no multi-bank PSUM reads on ACT.
"""

import sys

try:
    import concourse.bass as bass  # noqa: F401
except ImportError:
    sys.path.insert(0, "/opt/trn_rl_repo")

import numpy as np

import concourse.bass as bass
import concourse.mybir as mybir
import concourse.tile as tile
from concourse.bass import ts
from concourse.bass_utils import run_bass_kernel_spmd
from concourse.masks import make_identity

FP = mybir.dt.float32
FPR = mybir.dt.float32r
AF = mybir.ActivationFunctionType
ALU = mybir.AluOpType
AX = mybir.AxisListType

P = 128
C = 256
HW = 4096
HALF = 2048
NCH = 2          # channel chunks of 128
NJT = 32         # 128-wide key tiles
NIG = 4          # query i-groups of 512
NCHUNK = 8       # 1024-wide token chunks of the full image (stats)
NDMA = 8         # 512-wide DMA chunks per half-image row block
GROUPS = 32
GSIZE = C // GROUPS          # 8 channels per group
NELEM = GSIZE * HW           # 32768 elements per group
EPS = 1e-6


def _split_waits(nc, max_waits=1):
    """The pinned walrus rejects >1 sync-wait on ctrl instructions; hoist
    excess waits onto preceding NoOps on the same engine (same instruction
    stream, so ordering is preserved)."""
    ctr = 0
    for bb in nc.m.functions[0].blocks:
        out = []
        changed = False
        for inst in bb.instructions:
            si = getattr(inst, "sync_info", None)
            waits = list(si.on_wait) if (si and si.on_wait) else []
            if len(waits) > max_waits:
                changed = True
                head, rest = waits[:-max_waits], waits[-max_waits:]
                for k in range(0, len(head), max_waits):
                    ctr += 1
                    nop = mybir.InstNoOp(name=f"I-wsplit-{ctr}", ins=[], outs=[])
                    nop.engine = inst.engine
                    nop.sync_info = mybir.SyncInfo(
                        on_wait=head[k : k + max_waits], on_update=[]
                    )
                    out.append(nop)
                inst.sync_info = mybir.SyncInfo(
                    on_wait=rest, on_update=list(si.on_update or [])
                )
            out.append(inst)
        if changed:
            bb.instructions = out


def build_nc_fp8(split=True, reps=1, exp_dve=(2, (0,)), fused_dve=True,
                 pipe_stats=True, lookahead=4, nact_stats=1):
    F8 = mybir.dt.float8e4
    DRM = mybir.MatmulPerfMode.DoubleRow
    NJP = NJT // 2  # 16 key pair-tiles
    LOG2E = 1.4426950408889634
    LAM = 8.0 * LOG2E * (float(C) ** -0.5)  # score scale: sps = LAM * s_true
    ACT_SCALE = 1.0 / (8.0 * LOG2E)         # sps -> s_true for ACT exact exp
    MLN32 = -3.4657359027997265             # -ln(32)
    EXP8_B = 16.0 - 0.32                    # Schraudolph intercept (-0.32 centers)
    # stats chunking (per o-half, HW = 8 x 512): DVE bn_stats is HW-capped
    # at 512 free elements, ACT Square/Identity chunks run 1024-wide;
    # nact_stats = # of 1024-wide ACT pairs per o-half
    NDVE_ST = NCHUNK - 2 * nact_stats       # 512-wide chunks on DVE bn_stats

    nc = bass.Bass()
    xf = nc.dram_tensor("xf", [C, HW], FP, kind="ExternalInput")
    mT = nc.dram_tensor("mT", [C, C], FP, kind="ExternalInput")
    wvT = nc.dram_tensor("wvT", [C, C], FP, kind="ExternalInput")
    wpT = nc.dram_tensor("wpT", [C, C], FP, kind="ExternalInput")
    w2c = nc.dram_tensor("w2c", [P, NCH], FP, kind="ExternalInput")
    bp2 = nc.dram_tensor("bp2", [P, NCH], FP, kind="ExternalInput")
    gns = nc.dram_tensor("gns", [P, NCH], FP, kind="ExternalInput")
    gnb = nc.dram_tensor("gnb", [P, NCH], FP, kind="ExternalInput")
    gsel = nc.dram_tensor("gsel", [P, P], FP, kind="ExternalInput")
    y = nc.dram_tensor("y", [C, HALF], FP, kind="ExternalOutput")

    with tile.TileContext(nc) as tc:
        with (
            tc.tile_pool(name="wts", bufs=1) as wts,
            tc.tile_pool(name="big", bufs=1) as big,
            tc.tile_pool(name="upool", bufs=8) as upool,
            tc.tile_pool(name="attp", bufs=2) as attp,
            tc.tile_pool(name="small", bufs=3) as small,
            tc.tile_pool(name="stats", bufs=1) as stats,
            tc.tile_pool(name="outp", bufs=3) as outp,
            tc.tile_pool(name="psS", bufs=2, space="PSUM") as psS,      # 2x2 banks
            tc.tile_pool(name="psAcc", bufs=2, space="PSUM") as psAcc,  # 2 banks
            tc.tile_pool(name="psD", bufs=1, space="PSUM") as psD,      # 1 bank
            tc.tile_pool(name="psP", bufs=1, space="PSUM") as psP,      # 1 bank
        ):
            # ---------------- input image first (critical path), then constants
            # stored as f32r so the residual identity-matmul may read it
            # natively; fp32 consumers use xf_sb bitcast views
            xf_r = big.tile([P, NCH, HW], FPR, tag="xf")
            dma_engines = [nc.sync, nc.gpsimd, nc.scalar]
            for o in range(NCH):
                for t8 in range(NDMA):
                    eng = dma_engines[(o * NDMA + t8) % len(dma_engines)]
                    eng.dma_start(
                        out=xf_r[:, o, ts(t8, 512)],
                        in_=xf[o * P : (o + 1) * P, ts(t8, 512)].bitcast(FPR),
                    )
            xf_sb = xf_r.bitcast(FP)

            w_sb = {}
            for name, dram in (("mT", mT), ("wvT", wvT), ("wpT", wpT)):
                t = wts.tile([P, NCH, C], FP, tag=f"w_{name}")
                nc.sync.dma_start(
                    out=t, in_=dram.rearrange("(o p) c -> p o c", p=P)
                )
                w_sb[name] = t
            gsel_sb = wts.tile([P, P], FP, tag="gsel")
            nc.sync.dma_start(out=gsel_sb, in_=gsel[:, :])
            w2_sb = wts.tile([P, NCH], FP, tag="w2")
            nc.sync.dma_start(out=w2_sb, in_=w2c[:, :])
            bp_sb = wts.tile([P, NCH], FP, tag="bp")
            nc.sync.dma_start(out=bp_sb, in_=bp2[:, :])
            gns_sb = wts.tile([P, NCH], FP, tag="gns")
            nc.sync.dma_start(out=gns_sb, in_=gns[:, :])
            gnb_sb = wts.tile([P, NCH], FP, tag="gnb")
            nc.sync.dma_start(out=gnb_sb, in_=gnb[:, :])

            # fp8 all-0.5 stationary for the PE-side softmax-denominator:
            # u8 = exp/32 => den_ps = den/64; with acc_ps = acc/4 (v8 = 8v)
            # the normalize acc_ps*recip(den_ps) yields 16*att — the fp8
            # att scale the projection expects
            ones8 = wts.tile([P, 2, P], F8, tag="ones8")
            nc.vector.memset(ones8, 0.5)
            # residual identity pre-scaled x256 to compensate the fp8
            # projection operand scaling (wp8 = 16wp, att8 = 16att); the
            # ACT drain divides the whole PSUM by 256 before adding ob
            ident_fp = wts.tile([P, P], FP, tag="ident_fp")
            make_identity(nc, ident_fp)
            ident_fp256 = wts.tile([P, P], FP, tag="ident_fp256")
            nc.vector.tensor_scalar_mul(ident_fp256, ident_fp, 256.0)
            ident = wts.tile([P, P], FPR, tag="ident")
            nc.vector.tensor_copy(ident, ident_fp256)
            mln32 = wts.tile([P, 1], FP, tag="mln32")
            nc.vector.memset(mln32, MLN32)

            # fp8 projection weights, x16 so ~N(0,1/256) entries sit in
            # e4m3's normal range (hoisted: not seff-dependent)
            wp8 = wts.tile([P, NCH, C], F8, tag="wp8")
            for o in range(NCH):
                nc.vector.tensor_scalar_mul(wp8[:, o], w_sb["wpT"][:, o], 16.0)

            # fp8 copy of x (ACT + gpsimd + DVE so no engine serializes)
            x8 = big.tile([P, NCH, HW], F8, tag="x8")
            for o in range(NCH):
                for t8 in range(NDMA):
                    m = (o * NDMA + t8) % 4
                    if m in (0, 2):
                        nc.scalar.copy(
                            out=x8[:, o, ts(t8, 512)], in_=xf_sb[:, o, ts(t8, 512)]
                        )
                    elif m == 1:
                        nc.gpsimd.tensor_copy(
                            x8[:, o, ts(t8, 512)], xf_sb[:, o, ts(t8, 512)]
                        )
                    else:
                        nc.vector.tensor_copy(
                            x8[:, o, ts(t8, 512)], xf_sb[:, o, ts(t8, 512)]
                        )

            # ---- GroupNorm chunk statistics (1024-wide chunks), emitted
            # either at rep start or pipelined into the previous rep's
            # attention. DVE bn_stats gives (count, mean, M2) for even/odd
            # elements; ACT chunks use Square/Identity + accum_out.
            bnt = stats.tile([P, NCH, NDVE_ST, 6], FP, tag="bnt")
            sumc = stats.tile([P, NCH, nact_stats], FP, tag="sumc")
            sqc = stats.tile([P, NCH, nact_stats], FP, tag="sqc")

            def stats_thunks():
                def dve_op(t8, o):
                    return lambda: nc.vector.bn_stats(
                        out=bnt[:, o, t8, :], in_=xf_sb[:, o, ts(t8, 512)]
                    )

                def act_op(pr, o):
                    def f():
                        base = NDVE_ST * 512 + pr * 1024
                        sl = slice(base, base + 1024)
                        scr = small.tile([P, 1024], FP, tag="scr")
                        nc.scalar.activation(
                            out=scr, in_=xf_sb[:, o, sl],
                            func=AF.Square, accum_out=sqc[:, o, pr : pr + 1],
                        )
                        scr2 = small.tile([P, 1024], FP, tag="scr")
                        nc.scalar.activation(
                            out=scr2, in_=xf_sb[:, o, sl],
                            func=AF.Identity, accum_out=sumc[:, o, pr : pr + 1],
                        )
                    return f

                return [dve_op(t8, o) for t8 in range(NDVE_ST) for o in range(NCH)] + [
                    act_op(pr, o) for pr in range(nact_stats) for o in range(NCH)
                ]

            for _rep in range(reps):
                if _rep == 0 or not pipe_stats:
                    for f in stats_thunks():
                        f()

                # ---------------- phase 1a: reduce chunk stats ----------------
                seff = stats.tile([P, NCH], FP, tag="seff")
                beff = stats.tile([P, NCH], FP, tag="beff")
                eps_sb = stats.tile([P, 1], FP, tag="eps")
                nc.vector.memset(eps_sb, EPS)
                for o in range(NCH):
                    me = bnt[:, o, :, 1]
                    mo = bnt[:, o, :, 4]
                    m2e = bnt[:, o, :, 2]
                    m2o = bnt[:, o, :, 5]
                    tm = stats.tile([P, NDVE_ST], FP, tag=f"tm{o}")
                    nc.vector.tensor_tensor(tm, me, mo, ALU.add)
                    t2 = stats.tile([P, NDVE_ST], FP, tag=f"t2{o}")
                    nc.vector.tensor_tensor(t2, m2e, m2o, ALU.add)
                    sqm = stats.tile([P, 2 * NDVE_ST], FP, tag=f"sqm{o}")
                    nc.vector.tensor_tensor(sqm[:, 0:NDVE_ST], me, me, ALU.mult)
                    nc.vector.tensor_tensor(sqm[:, NDVE_ST:], mo, mo, ALU.mult)
                    red = stats.tile([P, 5], FP, tag=f"red{o}")
                    nc.vector.tensor_reduce(
                        out=red[:, 0:1], in_=tm, axis=AX.X, op=ALU.add
                    )
                    nc.vector.tensor_reduce(
                        out=red[:, 1:2], in_=t2, axis=AX.X, op=ALU.add
                    )
                    nc.vector.tensor_reduce(
                        out=red[:, 2:3], in_=sqm, axis=AX.X, op=ALU.add
                    )
                    nc.vector.tensor_reduce(
                        out=red[:, 3:4], in_=sumc[:, o], axis=AX.X, op=ALU.add
                    )
                    nc.vector.tensor_reduce(
                        out=red[:, 4:5], in_=sqc[:, o], axis=AX.X, op=ALU.add
                    )
                    part = stats.tile([P, 2], FP, tag=f"part{o}")
                    # bn_stats means are over 256 elements (even/odd of 512)
                    nc.vector.scalar_tensor_tensor(
                        part[:, 0:1], red[:, 0:1], 256.0, red[:, 3:4],
                        ALU.mult, ALU.add,
                    )
                    p1t = stats.tile([P, 1], FP, tag=f"p1t{o}")
                    nc.vector.scalar_tensor_tensor(
                        p1t, red[:, 2:3], 256.0, red[:, 1:2],
                        ALU.mult, ALU.add,
                    )
                    nc.vector.tensor_tensor(
                        part[:, 1:2], p1t, red[:, 4:5], ALU.add
                    )
                    gps = psD.tile([P, 512], FP, tag="psd", name="gps")[:, :2]
                    nc.tensor.matmul(gps, lhsT=gsel_sb, rhs=part, start=True, stop=True)
                    mean = stats.tile([P, 1], FP, tag=f"mean{o}")
                    nc.vector.tensor_scalar_mul(mean, gps[:, 0:1], 1.0 / NELEM)
                    ex2 = stats.tile([P, 1], FP, tag=f"ex2{o}")
                    nc.vector.tensor_scalar_mul(ex2, gps[:, 1:2], 1.0 / NELEM)
                    msq = stats.tile([P, 1], FP, tag=f"msq{o}")
                    nc.vector.tensor_mul(msq, mean, mean)
                    var = stats.tile([P, 1], FP, tag=f"var{o}")
                    nc.vector.tensor_tensor(var, ex2, msq, ALU.subtract)
                    # rstd = exp(-0.5 ln(var + eps)) — stays in the exp table set
                    lnv = stats.tile([P, 1], FP, tag=f"lnv{o}")
                    nc.scalar.activation(out=lnv, in_=var, func=AF.Ln, bias=eps_sb)
                    rstd = stats.tile([P, 1], FP, tag=f"rstd{o}")
                    nc.scalar.activation(out=rstd, in_=lnv, func=AF.Exp, scale=-0.5)
                    nc.vector.tensor_mul(seff[:, o : o + 1], gns_sb[:, o : o + 1], rstd)
                    tmp = stats.tile([P, 1], FP, tag=f"tmp{o}")
                    nc.vector.tensor_mul(tmp, mean, seff[:, o : o + 1])
                    nc.vector.tensor_tensor(
                        beff[:, o : o + 1], gnb_sb[:, o : o + 1], tmp, ALU.subtract
                    )

                # ---------------- phase 1b: folds ----------------
                def matvec(wname, rhs_sb, out_tile):
                    for oo in range(NCH):
                        mv = psD.tile([P, 512], FP, tag="psd", name="mv")[:, :1]
                        for oi in range(NCH):
                            lhs = w_sb[wname][:, oi, oo * P : (oo + 1) * P]
                            nc.tensor.matmul(
                                mv, lhsT=lhs, rhs=rhs_sb[:, oi : oi + 1],
                                start=(oi == 0), stop=(oi == NCH - 1),
                            )
                        nc.vector.tensor_copy(out_tile[:, oo : oo + 1], mv)

                vb = stats.tile([P, NCH], FP, tag="vb")
                pvb = stats.tile([P, NCH], FP, tag="pvb")
                ob = stats.tile([P, NCH], FP, tag="ob")
                mb = stats.tile([P, NCH], FP, tag="mb")
                matvec("wvT", beff, vb)
                matvec("wpT", vb, pvb)
                nc.vector.tensor_add(ob, pvb, bp_sb)
                matvec("mT", beff, mb)  # (mT)^T @ beff = M @ beff

                # Z-drain affine: z8 = A2 ⊙ zps + B2 with
                # A2 = LAM*seff/16, B2 = LAM*seff*(M@beff + wk^T bq)
                a2 = stats.tile([P, NCH], FP, tag="a2")
                nc.vector.tensor_scalar_mul(a2, seff, LAM / 16.0)
                b2t = stats.tile([P, NCH], FP, tag="b2t")
                nc.vector.tensor_add(b2t, mb, w2_sb)
                b2s = stats.tile([P, NCH], FP, tag="b2s")
                nc.vector.tensor_mul(b2s, b2t, seff)
                b2 = stats.tile([P, NCH], FP, tag="b2")
                nc.vector.tensor_scalar_mul(b2, b2s, LAM)

                seffv = stats.tile([P, NCH], FP, tag="seffv")
                nc.vector.tensor_scalar_mul(seffv, seff, 8.0)
                seff16 = stats.tile([P, NCH], FP, tag="seff16")
                nc.vector.tensor_scalar_mul(seff16, seff, 16.0)

                # fp8 weights: M8T = fp8(16 seff ⊙ M^T) (query-side GN fold),
                # w8v = fp8(8 seff ⊙ wv^T)
                m8 = wts.tile([P, NCH, C], F8, tag="m8", name="m8")
                w8v = wts.tile([P, NCH, C], F8, tag="w8v", name="w8v")
                for o in range(NCH):
                    nc.gpsimd.tensor_scalar_mul(
                        m8[:, o], w_sb["mT"][:, o], seff16[:, o : o + 1]
                    )
                    nc.gpsimd.tensor_scalar_mul(
                        w8v[:, o], w_sb["wvT"][:, o], seffv[:, o : o + 1]
                    )

                # ---------------- phase 1c: Z (per i-group) + V ----------------
                z8 = big.tile([P, NIG, NCH, 512], F8, tag="z8")

                def z_prod(g):
                    zps = psS.tile([P, 2, 512], FP, tag="pss", name=f"zps{g}")
                    for oj in range(NCH):
                        nc.tensor.matmul(
                            zps[:, oj, :],
                            lhsT=m8[:, :, oj * P : (oj + 1) * P],
                            rhs=x8[:, :, ts(g, 512)],
                            start=True, stop=True, perf_mode=DRM,
                        )
                    for oj in range(NCH):
                        if (g + oj) % 2 == 0:
                            nc.vector.tensor_scalar(
                                out=z8[:, g, oj, :], in0=zps[:, oj, :],
                                scalar1=a2[:, oj : oj + 1],
                                scalar2=b2[:, oj : oj + 1],
                                op0=ALU.mult, op1=ALU.add,
                            )
                        else:
                            nc.scalar.activation(
                                out=z8[:, g, oj, :], in_=zps[:, oj, :],
                                func=AF.Identity,
                                scale=a2[:, oj : oj + 1],
                                bias=b2[:, oj : oj + 1],
                            )

                v8 = big.tile([P, NJT, C], F8, tag="v8")

                def v_prod_pair(jp):
                    # two key-tiles' V into one PSUM bank, one cast out
                    # (gpsimd cannot read PSUM, so casts live on ACT/DVE)
                    vps = psP.tile([P, 2, 256], FP, tag="psp", name="psv")
                    for h in range(2):
                        nc.tensor.matmul(
                            vps[:, h, :],
                            lhsT=x8[:, :, (2 * jp + h) * P : (2 * jp + h + 1) * P],
                            rhs=w8v,
                            start=True, stop=True, perf_mode=DRM,
                        )
                    dst = v8[:, 2 * jp : 2 * jp + 2, :]
                    if jp % 2 == 0:
                        nc.scalar.copy(out=dst, in_=vps)
                    else:
                        nc.vector.tensor_copy(dst, vps)

                z_prod(0)

                # ---------------- phase 2: attention (channel-major) ----------------
                def scores_exp(g, jp):
                    sps = psS.tile([P, 2, 512], FP, tag="pss", name="sps")
                    u8t = upool.tile([P, 2, 512], F8, tag="u8")
                    idx = g * NJP + jp
                    on_dve = idx % exp_dve[0] in exp_dve[1]
                    for h in range(2):
                        j = 2 * jp + h
                        nc.tensor.matmul(
                            sps[:, h, :],
                            lhsT=x8[:, :, j * P : (j + 1) * P],
                            rhs=z8[:, g],
                            start=True, stop=True, perf_mode=DRM,
                        )
                    for h in range(2):
                        if on_dve and fused_dve:
                            # stock-op Schraudolph into fp8 bits, fully
                            # fused: the score scaling already makes the
                            # PSUM value the exponent-bits affine, so one
                            # 2-op tensor_scalar (add intercept, relu) with
                            # int8 output convert produces the fp8 bytes
                            nc.vector.tensor_scalar(
                                out=u8t[:, h, :].bitcast(mybir.dt.int8),
                                in0=sps[:, h, :],
                                scalar1=EXP8_B, scalar2=0.0,
                                op0=ALU.add, op1=ALU.max,
                            )
                        elif on_dve:
                            scr = small.tile([P, 512], FP, tag="sscr", name="scr")
                            nc.vector.tensor_scalar(
                                out=scr, in0=sps[:, h, :],
                                scalar1=1.0, scalar2=EXP8_B,
                                op0=ALU.mult, op1=ALU.add,
                            )
                            nc.vector.tensor_scalar_max(
                                u8t[:, h, :].bitcast(mybir.dt.int8), scr, 0.0
                            )
                        else:
                            nc.scalar.activation(
                                out=u8t[:, h, :], in_=sps[:, h, :],
                                func=AF.Exp, scale=ACT_SCALE, bias=mln32,
                            )
                    return u8t

                acc = {}
                dps = {}

                def pv_den(g, jp, u8t):
                    for oc in range(NCH):
                        nc.tensor.matmul(
                            acc[g][oc],
                            lhsT=v8[:, 2 * jp : 2 * jp + 2, oc * P : (oc + 1) * P],
                            rhs=u8t,
                            start=(jp == 0), stop=(jp == NJP - 1),
                            perf_mode=DRM,
                        )
                    nc.tensor.matmul(
                        dps[g],
                        lhsT=ones8,
                        rhs=u8t,
                        start=(jp == 0), stop=(jp == NJP - 1),
                        perf_mode=DRM,
                    )

                def epilogue(g):
                    att_g = attp.tile([P, NCH, 512], F8, tag="att", name=f"att{g}")
                    # normalize: DVE reciprocal (PSUM->SBUF) then per-half
                    # multiply (one PSUM operand per DVE op); lands 16*att
                    # in fp8 for the DoubleRow projection
                    recip = small.tile([P, 512], FP, tag="recip", name=f"rc{g}")
                    nc.vector.reciprocal(recip, dps[g])
                    for oc in range(NCH):
                        nc.vector.tensor_tensor(
                            att_g[:, oc, :], acc[g][oc], recip, ALU.mult
                        )
                    # fp8 DR projection (one pass per oo); the residual rides
                    # the accumulation group as a 256-scaled f32r identity-
                    # matmul; ACT drains with scale 1/256 then adds ob
                    for oo in range(NCH):
                        pps = psP.tile([P, 512], FP, tag="psp", name=f"pp{g}_{oo}")
                        nc.tensor.matmul(
                            pps,
                            lhsT=wp8[:, :, oo * P : (oo + 1) * P],
                            rhs=att_g,
                            start=True, stop=False, perf_mode=DRM,
                        )
                        nc.tensor.matmul(
                            pps,
                            lhsT=ident,
                            rhs=xf_r[:, oo, ts(g, 512)],
                            start=False, stop=True,
                        )
                        ot = outp.tile([P, 512], FP, tag="out", name=f"ot{g}_{oo}")
                        nc.scalar.activation(
                            out=ot, in_=pps, func=AF.Identity,
                            scale=1.0 / 256.0, bias=ob[:, oo : oo + 1],
                        )
                        oeng = dma_engines[(g * NCH + oo) % len(dma_engines)]
                        oeng.dma_start(
                            out=y[oo * P : (oo + 1) * P, ts(g, 512)], in_=ot
                        )

                # flat software pipeline over (g, jp): PV/den lag the
                # scores/exp stream by `lookahead` tiles, epilogues follow
                # their last PV, so the in-order PE always has independent
                # work queued ahead of any instruction that waits on ACT/DVE
                # results. V rides inside g=0; Z(g+1) is produced at (g, 2).
                # With reps>1, the NEXT rep's chunk stats are doled out one
                # per few tiles (x is rep-invariant), removing them from the
                # serial head.
                pending = stats_thunks() if (pipe_stats and _rep + 1 < reps) else []
                T = NIG * NJP
                u8s = {}
                for t in range(T + lookahead):
                    if t < T:
                        g, jp = divmod(t, NJP)
                        if g == 0:
                            if jp == 0:
                                v_prod_pair(0)
                            if jp + 1 < NJP:
                                v_prod_pair(jp + 1)
                        if jp == 2 and g + 1 < NIG:
                            z_prod(g + 1)
                        if jp == 0:
                            acc[g] = [
                                psAcc.tile(
                                    [P, 512], FP, tag="psa", name=f"acc{g}_{oc}"
                                )
                                for oc in range(NCH)
                            ]
                            dps[g] = psD.tile([P, 512], FP, tag="psd", name=f"dps{g}")
                        u8s[t] = scores_exp(g, jp)
                        if pending and t >= 10 and (t - 10) % 2 == 0:
                            pending.pop(0)()
                    if t >= lookahead:
                        t2 = t - lookahead
                        g2, jp2 = divmod(t2, NJP)
                        pv_den(g2, jp2, u8s.pop(t2))
                        if jp2 == NJP - 1:
                            epilogue(g2)
                assert not pending, f"{len(pending)} stats thunks unplaced"

    if split:
        _split_waits(nc)
    return nc


BUILD = build_nc_fp8

_NC_CACHE = None


def _get_nc():
    global _NC_CACHE
    if _NC_CACHE is None:
        _NC_CACHE = build_nc_fp8()
    return _NC_CACHE


def make_in_maps(x, gn_scale, gn_bias, wq, bq, wk, bk, wv, bv, wp, bp):
    B = x.shape[0]
    f32 = np.float32

    def col2(v):  # [256] -> [128, 2], column o = channels o*128..o*128+127
        return np.ascontiguousarray(np.asarray(v, f32).reshape(NCH, P).T)

    wq = np.asarray(wq, f32)
    wk = np.asarray(wk, f32)
    M = wk.T @ wq                      # scores bilinear form: S = of^T M of
    mT = np.ascontiguousarray(M.T)
    w2cv = col2(wk.T @ np.asarray(bq, f32))
    wvT = np.ascontiguousarray(np.asarray(wv, f32).T)
    wpT = np.ascontiguousarray(np.asarray(wp, f32).T)
    bp2 = col2(np.asarray(wp, f32) @ np.asarray(bv, f32) + np.asarray(bp, f32))
    gns = col2(gn_scale)
    gnb = col2(gn_bias)
    gsel = np.kron(np.eye(P // GSIZE, dtype=f32), np.ones((GSIZE, GSIZE), f32))

    xr = np.asarray(x, f32).reshape(B, C, HW)
    in_maps = []
    for core in range(8):
        b, h = core // 2, core % 2
        # rotate so this core's query half sits at columns 0..HALF-1
        # (attention/groupnorm are token-order invariant)
        xfb = np.ascontiguousarray(np.roll(xr[b], -h * HALF, axis=1))
        in_maps.append(
            {
                "xf": xfb,
                "mT": mT, "wvT": wvT, "wpT": wpT,
                "w2c": w2cv, "bp2": bp2, "gns": gns, "gnb": gnb, "gsel": gsel,
            }
        )
    return in_maps


def assemble_out(results, B=4):
    out = np.empty((B, C, HW), np.float32)
    for core in range(8):
        b, h = core // 2, core % 2
        out[b, :, h * HALF : (h + 1) * HALF] = results[core]["y"]
    return out.reshape(B, C, 64, 64)


def kernel(**inputs):
    in_maps = make_in_maps(**inputs)
    try:
        nc = _get_nc()
        res = run_bass_kernel_spmd(nc, in_maps, list(range(8)))
    except Exception:
        # let a possibly-wedged device recover, then retry once
        import time as _time

        _time.sleep(90)
        nc = _get_nc()
        res = run_bass_kernel_spmd(nc, in_maps, list(range(8)))
    return assemble_out(res.results, B=inputs["x"].shape[0])


if __name__ == "__main__":
    rng = np.random.default_rng(0)
    ins = {
        "x": rng.standard_normal((4, C, 64, 64)).astype(np.float32),
        "gn_scale": np.ones(C, np.float32),
        "gn_bias": np.zeros(C, np.float32),
    }
    for w in ("wq", "wk", "wv", "wp"):
        ins[w] = (rng.standard_normal((C, C)) / 16.0).astype(np.float32)
    for b in ("bq", "bk", "bv", "bp"):
        ins[b] = np.zeros(C, np.float32)
    out = kernel(**ins)
    print(out.shape, out.dtype, float(np.abs(out).mean()))


# revision 51
# speedup vs baseline: 1.3971x; 1.3246x over previous
"""AttentionBlock (GroupNorm + single-head self-attention + proj + residual)
for Trainium2, 8 NeuronCores, data-parallel over (batch, token-half).

Shapes (hardcoded): x [4, 256, 64, 64] fp32, weights [256, 256] fp32.
Each core handles one (batch b, token-half h): 2048 query tokens against the
full 4096 keys/values of its batch, entirely in SBUF.

v3 "Z-scheme" kernel (build_nc_fp8): the scores bilinear form is
S = of^T (wk^T wq) of, so the host precomputes M = wk^T wq (weight-only
preprocessing) and the kernel never materializes K or Q at all:

  - Z[g] = M8T^T @ x8[queries of i-group g]   (2 fp8-DR passes per g)
    with M8T = fp8(16 * seff ⊙ M^T) folding the query-side GroupNorm
    scale; the drain z8 = A2 ⊙ zps + B2 folds the key-side seff, the
    query bias (w2 = wk^T bq, host-precomputed) and M @ beff.
  - S^T tile = x8_keytile^T @ z8  (keys on partitions, fp8 DR),
    identical cost to the old K^T Q but with NO K/Q production matmuls
    and NO k8/q8 PSUM drains (-16 PE passes, -40 vector-engine drains).
  - Score scaling LAM = 8*log2e*C^-0.5 makes the PSUM score the
    Schraudolph exponent-bits affine: the DVE exp is ONE fused op
    i8 = max(sps + B, 0) with int8 convert whose bits ARE fp8(exp/32);
    ACT tiles use the exact exp (scale 1/(8 log2e), bias -ln32).
  - u = exp/32, v8 = 8v, ones = 0.5: acc_ps = acc/4, den_ps = den/64,
    so acc_ps/den_ps = 16*att — the fp8 scale the DR projection expects
    with wp8 = fp8(16 wp); the 256x is removed in the ACT drain (the
    residual identity-matmul is pre-scaled by 256).
  - GroupNorm stats: 1024-wide chunks, split DVE bn_stats / ACT
    Square+Identity accum (all funcs share one ACT table set); with
    reps>1 the next rep's chunk stats are emitted DURING this rep's
    attention (x is rep-invariant), removing them from the serial head.
  - HW-validated constraints respected: gpsimd never touches PSUM; one
    PSUM operand per DVE op; no multi-bank PSUM reads on ACT.
"""

import sys

try:
    import concourse.bass as bass  # noqa: F401
except ImportError:
    sys.path.insert(0, "/opt/trn_rl_repo")

import numpy as np

import concourse.bass as bass
import concourse.mybir as mybir
import concourse.tile as tile
from concourse.bass import ts
from concourse.bass_utils import run_bass_kernel_spmd
from concourse.masks import make_identity

FP = mybir.dt.float32
FPR = mybir.dt.float32r
AF = mybir.ActivationFunctionType
ALU = mybir.AluOpType
AX = mybir.AxisListType

P = 128
C = 256
HW = 4096
HALF = 2048
NCH = 2          # channel chunks of 128
NJT = 32         # 128-wide key tiles
NIG = 4          # query i-groups of 512
NCHUNK = 8       # 1024-wide token chunks of the full image (stats)
NDMA = 8         # 512-wide DMA chunks per half-image row block
GROUPS = 32
GSIZE = C // GROUPS          # 8 channels per group
NELEM = GSIZE * HW           # 32768 elements per group
EPS = 1e-6


def _split_waits(nc, max_waits=1):
    """The pinned walrus rejects >1 sync-wait on ctrl instructions; hoist
    excess waits onto preceding NoOps on the same engine (same instruction
    stream, so ordering is preserved)."""
    ctr = 0
    for bb in nc.m.functions[0].blocks:
        out = []
        changed = False
        for inst in bb.instructions:
            si = getattr(inst, "sync_info", None)
            waits = list(si.on_wait) if (si and si.on_wait) else []
            if len(waits) > max_waits:
                changed = True
                head, rest = waits[:-max_waits], waits[-max_waits:]
                for k in range(0, len(head), max_waits):
                    ctr += 1
                    nop = mybir.InstNoOp(name=f"I-wsplit-{ctr}", ins=[], outs=[])
                    nop.engine = inst.engine
                    nop.sync_info = mybir.SyncInfo(
                        on_wait=head[k : k + max_waits], on_update=[]
                    )
                    out.append(nop)
                inst.sync_info = mybir.SyncInfo(
                    on_wait=rest, on_update=list(si.on_update or [])
                )
            out.append(inst)
        if changed:
            bb.instructions = out


def build_nc_fp8(split=True, reps=1, exp_dve=(2, (0,)), fused_dve=True,
                 pipe_stats=True, lookahead=4, nact_stats=1, z_on_act=True,
                 npool_stats=0, exp1024=True):
    F8 = mybir.dt.float8e4
    DRM = mybir.MatmulPerfMode.DoubleRow
    NJP = NJT // 2  # 16 key pair-tiles
    LOG2E = 1.4426950408889634
    LAM = 8.0 * LOG2E * (float(C) ** -0.5)  # score scale: sps = LAM * s_true
    ACT_SCALE = 1.0 / (8.0 * LOG2E)         # sps -> s_true for ACT exact exp
    MLN32 = -3.4657359027997265             # -ln(32)
    EXP8_B = 16.0 - 0.32                    # Schraudolph intercept (-0.32 centers)
    # stats chunking (per o-half, HW = 8 x 512): DVE bn_stats is HW-capped
    # at 512 free elements; ACT Square/Identity chunks run 1024-wide
    # (nact_stats pairs); npool_stats 512-chunks go to gpsimd as
    # square + two reduces (gpsimd reads SBUF only — legal here)
    NDVE_ST = NCHUNK - 2 * nact_stats - npool_stats

    nc = bass.Bass()
    xf = nc.dram_tensor("xf", [C, HW], FP, kind="ExternalInput")
    mT = nc.dram_tensor("mT", [C, C], FP, kind="ExternalInput")
    wvT = nc.dram_tensor("wvT", [C, C], FP, kind="ExternalInput")
    wpT = nc.dram_tensor("wpT", [C, C], FP, kind="ExternalInput")
    w2c = nc.dram_tensor("w2c", [P, NCH], FP, kind="ExternalInput")
    bp2 = nc.dram_tensor("bp2", [P, NCH], FP, kind="ExternalInput")
    gns = nc.dram_tensor("gns", [P, NCH], FP, kind="ExternalInput")
    gnb = nc.dram_tensor("gnb", [P, NCH], FP, kind="ExternalInput")
    gsel = nc.dram_tensor("gsel", [P, P], FP, kind="ExternalInput")
    y = nc.dram_tensor("y", [C, HALF], FP, kind="ExternalOutput")

    with tile.TileContext(nc) as tc:
        with (
            tc.tile_pool(name="wts", bufs=1) as wts,
            tc.tile_pool(name="big", bufs=1) as big,
            tc.tile_pool(name="upool", bufs=8) as upool,
            tc.tile_pool(name="attp", bufs=2) as attp,
            tc.tile_pool(name="small", bufs=3) as small,
            tc.tile_pool(name="stats", bufs=1) as stats,
            tc.tile_pool(name="outp", bufs=3) as outp,
            tc.tile_pool(name="psS", bufs=2, space="PSUM") as psS,      # 2x2 banks
            tc.tile_pool(name="psAcc", bufs=1, space="PSUM") as psAcc,  # 2 banks
            tc.tile_pool(name="psD", bufs=1, space="PSUM") as psD,      # 1 bank
            tc.tile_pool(name="psP", bufs=1, space="PSUM") as psP,      # 1 bank
        ):
            # ---------------- input image first (critical path), then constants
            # stored as f32r so the residual identity-matmul may read it
            # natively; fp32 consumers use xf_sb bitcast views
            xf_r = big.tile([P, NCH, HW], FPR, tag="xf")
            dma_engines = [nc.sync, nc.gpsimd, nc.scalar]
            for o in range(NCH):
                for t8 in range(NDMA):
                    eng = dma_engines[(o * NDMA + t8) % len(dma_engines)]
                    eng.dma_start(
                        out=xf_r[:, o, ts(t8, 512)],
                        in_=xf[o * P : (o + 1) * P, ts(t8, 512)].bitcast(FPR),
                    )
            xf_sb = xf_r.bitcast(FP)

            w_sb = {}
            for name, dram in (("mT", mT), ("wvT", wvT), ("wpT", wpT)):
                t = wts.tile([P, NCH, C], FP, tag=f"w_{name}")
                nc.sync.dma_start(
                    out=t, in_=dram.rearrange("(o p) c -> p o c", p=P)
                )
                w_sb[name] = t
            gsel_sb = wts.tile([P, P], FP, tag="gsel")
            nc.sync.dma_start(out=gsel_sb, in_=gsel[:, :])
            w2_sb = wts.tile([P, NCH], FP, tag="w2")
            nc.sync.dma_start(out=w2_sb, in_=w2c[:, :])
            bp_sb = wts.tile([P, NCH], FP, tag="bp")
            nc.sync.dma_start(out=bp_sb, in_=bp2[:, :])
            gns_sb = wts.tile([P, NCH], FP, tag="gns")
            nc.sync.dma_start(out=gns_sb, in_=gns[:, :])
            gnb_sb = wts.tile([P, NCH], FP, tag="gnb")
            nc.sync.dma_start(out=gnb_sb, in_=gnb[:, :])

            # fp8 all-0.5 stationary for the PE-side softmax-denominator:
            # u8 = exp/32 => den_ps = den/64; with acc_ps = acc/4 (v8 = 8v)
            # the normalize acc_ps*recip(den_ps) yields 16*att — the fp8
            # att scale the projection expects
            ones8 = wts.tile([P, 2, P], F8, tag="ones8")
            nc.vector.memset(ones8, 0.5)
            # residual identity pre-scaled x256 to compensate the fp8
            # projection operand scaling (wp8 = 16wp, att8 = 16att); the
            # ACT drain divides the whole PSUM by 256 before adding ob
            ident_fp = wts.tile([P, P], FP, tag="ident_fp")
            make_identity(nc, ident_fp)
            ident_fp256 = wts.tile([P, P], FP, tag="ident_fp256")
            nc.vector.tensor_scalar_mul(ident_fp256, ident_fp, 256.0)
            ident = wts.tile([P, P], FPR, tag="ident")
            nc.vector.tensor_copy(ident, ident_fp256)
            mln32 = wts.tile([P, 1], FP, tag="mln32")
            nc.vector.memset(mln32, MLN32)

            # fp8 projection weights, x16 so ~N(0,1/256) entries sit in
            # e4m3's normal range (hoisted: not seff-dependent)
            wp8 = wts.tile([P, NCH, C], F8, tag="wp8")
            for o in range(NCH):
                nc.vector.tensor_scalar_mul(wp8[:, o], w_sb["wpT"][:, o], 16.0)

            # fp8 copy of x (ACT + gpsimd + DVE so no engine serializes)
            x8 = big.tile([P, NCH, HW], F8, tag="x8")
            for o in range(NCH):
                for t8 in range(NDMA):
                    m = (o * NDMA + t8) % 4
                    if m in (0, 2):
                        nc.scalar.copy(
                            out=x8[:, o, ts(t8, 512)], in_=xf_sb[:, o, ts(t8, 512)]
                        )
                    elif m == 1:
                        nc.gpsimd.tensor_copy(
                            x8[:, o, ts(t8, 512)], xf_sb[:, o, ts(t8, 512)]
                        )
                    else:
                        nc.vector.tensor_copy(
                            x8[:, o, ts(t8, 512)], xf_sb[:, o, ts(t8, 512)]
                        )

            # ---- GroupNorm chunk statistics (1024-wide chunks), emitted
            # either at rep start or pipelined into the previous rep's
            # attention. DVE bn_stats gives (count, mean, M2) for even/odd
            # elements; ACT chunks use Square/Identity + accum_out.
            bnt = stats.tile([P, NCH, NDVE_ST, 6], FP, tag="bnt")
            sumc = stats.tile([P, NCH, nact_stats], FP, tag="sumc")
            sqc = stats.tile([P, NCH, nact_stats], FP, tag="sqc")
            NPO = max(npool_stats, 1)
            psumc = stats.tile([P, NCH, NPO], FP, tag="psumc")
            psqc = stats.tile([P, NCH, NPO], FP, tag="psqc")

            def stats_thunks():
                def dve_op(t8, o):
                    return lambda: nc.vector.bn_stats(
                        out=bnt[:, o, t8, :], in_=xf_sb[:, o, ts(t8, 512)]
                    )

                def act_op(pr, o):
                    def f():
                        base = NDVE_ST * 512 + pr * 1024
                        sl = slice(base, base + 1024)
                        scr = small.tile([P, 1024], FP, tag="scr")
                        nc.scalar.activation(
                            out=scr, in_=xf_sb[:, o, sl],
                            func=AF.Square, accum_out=sqc[:, o, pr : pr + 1],
                        )
                        scr2 = small.tile([P, 1024], FP, tag="scr")
                        nc.scalar.activation(
                            out=scr2, in_=xf_sb[:, o, sl],
                            func=AF.Identity, accum_out=sumc[:, o, pr : pr + 1],
                        )
                    return f

                def pool_op(pi, o):
                    def f():
                        base = (NDVE_ST + 2 * nact_stats) * 512 + pi * 512
                        sl = slice(base, base + 512)
                        scr = small.tile([P, 512], FP, tag="pscr")
                        nc.gpsimd.tensor_scalar(
                            out=scr, in0=xf_sb[:, o, sl], scalar1=1.0,
                            scalar2=None, op0=ALU.mult,
                            accum_out=psumc[:, o, pi : pi + 1],
                        )
                        scr2 = small.tile([P, 512], FP, tag="pscr")
                        nc.gpsimd.scalar_tensor_tensor(
                            scr2, xf_sb[:, o, sl], 1.0, xf_sb[:, o, sl],
                            ALU.mult, ALU.mult,
                            accum_out=psqc[:, o, pi : pi + 1],
                        )
                    return f

                return (
                    [dve_op(t8, o) for t8 in range(NDVE_ST) for o in range(NCH)]
                    + [act_op(pr, o) for pr in range(nact_stats) for o in range(NCH)]
                    + [pool_op(pi, o) for pi in range(npool_stats) for o in range(NCH)]
                )

            # per-o pre-reduction of chunk stats down to the 2-column `part`
            # (pipelinable into the previous rep's attention; everything
            # after needs the PE gsel matmul whose PSUM bank is busy there)
            partt = {}
            for o in range(NCH):
                partt[o] = stats.tile(
                    [P, 2], FP, tag=f"part{o}", name=f"part{o}"
                )

            def chain_pre(o):
                me = bnt[:, o, :, 1]
                mo = bnt[:, o, :, 4]
                m2e = bnt[:, o, :, 2]
                m2o = bnt[:, o, :, 5]
                tm = stats.tile([P, NDVE_ST], FP, tag=f"tm{o}")
                nc.vector.tensor_tensor(tm, me, mo, ALU.add)
                t2 = stats.tile([P, NDVE_ST], FP, tag=f"t2{o}")
                nc.vector.tensor_tensor(t2, m2e, m2o, ALU.add)
                sqm = stats.tile([P, 2 * NDVE_ST], FP, tag=f"sqm{o}")
                nc.vector.tensor_tensor(sqm[:, 0:NDVE_ST], me, me, ALU.mult)
                nc.vector.tensor_tensor(sqm[:, NDVE_ST:], mo, mo, ALU.mult)
                red = stats.tile([P, 7], FP, tag=f"red{o}")
                nc.vector.tensor_reduce(
                    out=red[:, 0:1], in_=tm, axis=AX.X, op=ALU.add
                )
                nc.vector.tensor_reduce(
                    out=red[:, 1:2], in_=t2, axis=AX.X, op=ALU.add
                )
                nc.vector.tensor_reduce(
                    out=red[:, 2:3], in_=sqm, axis=AX.X, op=ALU.add
                )
                nc.vector.tensor_reduce(
                    out=red[:, 3:4], in_=sumc[:, o], axis=AX.X, op=ALU.add
                )
                nc.vector.tensor_reduce(
                    out=red[:, 4:5], in_=sqc[:, o], axis=AX.X, op=ALU.add
                )
                if npool_stats:
                    nc.vector.tensor_reduce(
                        out=red[:, 5:6], in_=psumc[:, o], axis=AX.X, op=ALU.add
                    )
                    nc.vector.tensor_reduce(
                        out=red[:, 6:7], in_=psqc[:, o], axis=AX.X, op=ALU.add
                    )
                    nc.vector.tensor_tensor(
                        red[:, 3:4], red[:, 3:4], red[:, 5:6], ALU.add
                    )
                    nc.vector.tensor_tensor(
                        red[:, 4:5], red[:, 4:5], red[:, 6:7], ALU.add
                    )
                # bn_stats means are over 256 elements (even/odd of 512)
                nc.vector.scalar_tensor_tensor(
                    partt[o][:, 0:1], red[:, 0:1], 256.0, red[:, 3:4],
                    ALU.mult, ALU.add,
                )
                p1t = stats.tile([P, 1], FP, tag=f"p1t{o}")
                nc.vector.scalar_tensor_tensor(
                    p1t, red[:, 2:3], 256.0, red[:, 1:2],
                    ALU.mult, ALU.add,
                )
                nc.vector.tensor_tensor(
                    partt[o][:, 1:2], p1t, red[:, 4:5], ALU.add
                )

            pipelined_pre = False
            for _rep in range(reps):
                if not pipelined_pre:
                    for f in stats_thunks():
                        f()
                    for o in range(NCH):
                        chain_pre(o)

                # ---------------- phase 1a: reduce chunk stats ----------------
                seff = stats.tile([P, NCH], FP, tag="seff")
                beff = stats.tile([P, NCH], FP, tag="beff")
                eps_sb = stats.tile([P, 1], FP, tag="eps")
                nc.vector.memset(eps_sb, EPS)
                for o in range(NCH):
                    part = partt[o]
                    gps = psD.tile([P, 512], FP, tag="psd", name="gps")[:, :2]
                    nc.tensor.matmul(gps, lhsT=gsel_sb, rhs=part, start=True, stop=True)
                    mean = stats.tile([P, 1], FP, tag=f"mean{o}")
                    nc.vector.tensor_scalar_mul(mean, gps[:, 0:1], 1.0 / NELEM)
                    ex2 = stats.tile([P, 1], FP, tag=f"ex2{o}")
                    nc.vector.tensor_scalar_mul(ex2, gps[:, 1:2], 1.0 / NELEM)
                    msq = stats.tile([P, 1], FP, tag=f"msq{o}")
                    nc.vector.tensor_mul(msq, mean, mean)
                    var = stats.tile([P, 1], FP, tag=f"var{o}")
                    nc.vector.tensor_tensor(var, ex2, msq, ALU.subtract)
                    # rstd = exp(-0.5 ln(var + eps)) — stays in the exp table set
                    lnv = stats.tile([P, 1], FP, tag=f"lnv{o}")
                    nc.scalar.activation(out=lnv, in_=var, func=AF.Ln, bias=eps_sb)
                    rstd = stats.tile([P, 1], FP, tag=f"rstd{o}")
                    nc.scalar.activation(out=rstd, in_=lnv, func=AF.Exp, scale=-0.5)
                    nc.vector.tensor_mul(seff[:, o : o + 1], gns_sb[:, o : o + 1], rstd)
                    tmp = stats.tile([P, 1], FP, tag=f"tmp{o}")
                    nc.vector.tensor_mul(tmp, mean, seff[:, o : o + 1])
                    nc.vector.tensor_tensor(
                        beff[:, o : o + 1], gnb_sb[:, o : o + 1], tmp, ALU.subtract
                    )

                # ---------------- phase 1b: folds ----------------
                def matvec(wname, rhs_sb, out_tile):
                    for oo in range(NCH):
                        mv = psD.tile([P, 512], FP, tag="psd", name="mv")[:, :1]
                        for oi in range(NCH):
                            lhs = w_sb[wname][:, oi, oo * P : (oo + 1) * P]
                            nc.tensor.matmul(
                                mv, lhsT=lhs, rhs=rhs_sb[:, oi : oi + 1],
                                start=(oi == 0), stop=(oi == NCH - 1),
                            )
                        nc.vector.tensor_copy(out_tile[:, oo : oo + 1], mv)

                vb = stats.tile([P, NCH], FP, tag="vb")
                pvb = stats.tile([P, NCH], FP, tag="pvb")
                ob = stats.tile([P, NCH], FP, tag="ob")
                mb = stats.tile([P, NCH], FP, tag="mb")
                matvec("wvT", beff, vb)
                matvec("wpT", vb, pvb)
                nc.vector.tensor_add(ob, pvb, bp_sb)
                matvec("mT", beff, mb)  # (mT)^T @ beff = M @ beff

                # Z-drain affine: z8 = A2 ⊙ zps + B2 with
                # A2 = LAM*seff/16, B2 = LAM*seff*(M@beff + wk^T bq)
                a2 = stats.tile([P, NCH], FP, tag="a2")
                nc.vector.tensor_scalar_mul(a2, seff, LAM / 16.0)
                b2t = stats.tile([P, NCH], FP, tag="b2t")
                nc.vector.tensor_add(b2t, mb, w2_sb)
                b2s = stats.tile([P, NCH], FP, tag="b2s")
                nc.vector.tensor_mul(b2s, b2t, seff)
                b2 = stats.tile([P, NCH], FP, tag="b2")
                nc.vector.tensor_scalar_mul(b2, b2s, LAM)

                seffv = stats.tile([P, NCH], FP, tag="seffv")
                nc.vector.tensor_scalar_mul(seffv, seff, 8.0)
                seff16 = stats.tile([P, NCH], FP, tag="seff16")
                nc.vector.tensor_scalar_mul(seff16, seff, 16.0)

                # fp8 weights: M8T = fp8(16 seff ⊙ M^T) (query-side GN fold),
                # w8v = fp8(8 seff ⊙ wv^T)
                m8 = wts.tile([P, NCH, C], F8, tag="m8", name="m8")
                w8v = wts.tile([P, NCH, C], F8, tag="w8v", name="w8v")
                for o in range(NCH):
                    nc.gpsimd.tensor_scalar_mul(
                        m8[:, o], w_sb["mT"][:, o], seff16[:, o : o + 1]
                    )
                    nc.gpsimd.tensor_scalar_mul(
                        w8v[:, o], w_sb["wvT"][:, o], seffv[:, o : o + 1]
                    )

                # ---------------- phase 1c: Z (per i-group) + V ----------------
                z8 = big.tile([P, NIG, NCH, 512], F8, tag="z8")

                def z_prod(g):
                    zps = psS.tile([P, 2, 512], FP, tag="pss", name=f"zps{g}")
                    for oj in range(NCH):
                        nc.tensor.matmul(
                            zps[:, oj, :],
                            lhsT=m8[:, :, oj * P : (oj + 1) * P],
                            rhs=x8[:, :, ts(g, 512)],
                            start=True, stop=True, perf_mode=DRM,
                        )
                    for oj in range(NCH):
                        if not z_on_act and (g + oj) % 2 == 0:
                            nc.vector.tensor_scalar(
                                out=z8[:, g, oj, :], in0=zps[:, oj, :],
                                scalar1=a2[:, oj : oj + 1],
                                scalar2=b2[:, oj : oj + 1],
                                op0=ALU.mult, op1=ALU.add,
                            )
                        else:
                            nc.scalar.activation(
                                out=z8[:, g, oj, :], in_=zps[:, oj, :],
                                func=AF.Identity,
                                scale=a2[:, oj : oj + 1],
                                bias=b2[:, oj : oj + 1],
                            )

                v8 = big.tile([P, NJT, C], F8, tag="v8")

                def v_prod_pair(jp):
                    # two key-tiles' V into one PSUM bank, one cast out
                    # (gpsimd cannot read PSUM, so casts live on ACT/DVE)
                    vps = psP.tile([P, 2, 256], FP, tag="psp", name="psv")
                    for h in range(2):
                        nc.tensor.matmul(
                            vps[:, h, :],
                            lhsT=x8[:, :, (2 * jp + h) * P : (2 * jp + h + 1) * P],
                            rhs=w8v,
                            start=True, stop=True, perf_mode=DRM,
                        )
                    dst = v8[:, 2 * jp : 2 * jp + 2, :]
                    if jp % 2 == 0:
                        nc.scalar.copy(out=dst, in_=vps)
                    else:
                        nc.vector.tensor_copy(dst, vps)

                z_prod(0)

                # ---------------- phase 2: attention (channel-major) ----------------
                def scores_exp(g, jp):
                    sps = psS.tile([P, 2, 512], FP, tag="pss", name="sps")
                    u8t = upool.tile([P, 2, 512], F8, tag="u8")
                    idx = g * NJP + jp
                    on_dve = idx % exp_dve[0] in exp_dve[1]
                    for h in range(2):
                        j = 2 * jp + h
                        nc.tensor.matmul(
                            sps[:, h, :],
                            lhsT=x8[:, :, j * P : (j + 1) * P],
                            rhs=z8[:, g],
                            start=True, stop=True, perf_mode=DRM,
                        )
                    if on_dve and fused_dve and exp1024:
                        # one fused op across both PSUM banks (DVE has no
                        # multi-bank read penalty, unlike ACT)
                        nc.vector.tensor_scalar(
                            out=u8t[:, :, :].bitcast(mybir.dt.int8),
                            in0=sps[:, :, :],
                            scalar1=EXP8_B, scalar2=0.0,
                            op0=ALU.add, op1=ALU.max,
                        )
                        return u8t
                    for h in range(2):
                        if on_dve and fused_dve:
                            # stock-op Schraudolph into fp8 bits, fully
                            # fused: the score scaling already makes the
                            # PSUM value the exponent-bits affine, so one
                            # 2-op tensor_scalar (add intercept, relu) with
                            # int8 output convert produces the fp8 bytes
                            nc.vector.tensor_scalar(
                                out=u8t[:, h, :].bitcast(mybir.dt.int8),
                                in0=sps[:, h, :],
                                scalar1=EXP8_B, scalar2=0.0,
                                op0=ALU.add, op1=ALU.max,
                            )
                        elif on_dve:
                            scr = small.tile([P, 512], FP, tag="sscr", name="scr")
                            nc.vector.tensor_scalar(
                                out=scr, in0=sps[:, h, :],
                                scalar1=1.0, scalar2=EXP8_B,
                                op0=ALU.mult, op1=ALU.add,
                            )
                            nc.vector.tensor_scalar_max(
                                u8t[:, h, :].bitcast(mybir.dt.int8), scr, 0.0
                            )
                        else:
                            nc.scalar.activation(
                                out=u8t[:, h, :], in_=sps[:, h, :],
                                func=AF.Exp, scale=ACT_SCALE, bias=mln32,
                            )
                    return u8t

                acc = {}
                dps = {}

                def pv_den(g, jp, u8t):
                    for oc in range(NCH):
                        nc.tensor.matmul(
                            acc[g][:, oc, :],
                            lhsT=v8[:, 2 * jp : 2 * jp + 2, oc * P : (oc + 1) * P],
                            rhs=u8t,
                            start=(jp == 0), stop=(jp == NJP - 1),
                            perf_mode=DRM,
                        )
                    nc.tensor.matmul(
                        dps[g],
                        lhsT=ones8,
                        rhs=u8t,
                        start=(jp == 0), stop=(jp == NJP - 1),
                        perf_mode=DRM,
                    )

                def epilogue(g):
                    att_g = attp.tile([P, NCH, 512], F8, tag="att", name=f"att{g}")
                    # normalize: DVE reciprocal (PSUM->SBUF) then per-half
                    # multiply (one PSUM operand per DVE op); lands 16*att
                    # in fp8 for the DoubleRow projection
                    recip = small.tile([P, 512], FP, tag="recip", name=f"rc{g}")
                    nc.vector.reciprocal(recip, dps[g])
                    # single 1024-wide normalize across both acc banks (DVE
                    # multi-bank PSUM reads are fine, unlike ACT); recip is
                    # broadcast over the oc dim via a 0-stride view
                    nc.vector.tensor_tensor(
                        att_g[:, :, :],
                        acc[g],
                        recip.rearrange("p (a q) -> p a q", a=1).broadcast_to(
                            (P, 2, 512)
                        ),
                        ALU.mult,
                    )
                    # fp8 DR projection (one pass per oo); the residual rides
                    # the accumulation group as a 256-scaled f32r identity-
                    # matmul; ACT drains with scale 1/256 then adds ob
                    for oo in range(NCH):
                        pps = psP.tile([P, 512], FP, tag="psp", name=f"pp{g}_{oo}")
                        nc.tensor.matmul(
                            pps,
                            lhsT=wp8[:, :, oo * P : (oo + 1) * P],
                            rhs=att_g,
                            start=True, stop=False, perf_mode=DRM,
                        )
                        nc.tensor.matmul(
                            pps,
                            lhsT=ident,
                            rhs=xf_r[:, oo, ts(g, 512)],
                            start=False, stop=True,
                        )
                        ot = outp.tile([P, 512], FP, tag="out", name=f"ot{g}_{oo}")
                        nc.scalar.activation(
                            out=ot, in_=pps, func=AF.Identity,
                            scale=1.0 / 256.0, bias=ob[:, oo : oo + 1],
                        )
                        # keep gpsimd out of the output-DMA path: a slow Pool
                        # compute op would head-of-line block the descriptor
                        oeng = (nc.sync, nc.scalar)[(g * NCH + oo) % 2]
                        oeng.dma_start(
                            out=y[oo * P : (oo + 1) * P, ts(g, 512)], in_=ot
                        )

                # flat software pipeline over (g, jp): PV/den lag the
                # scores/exp stream by `lookahead` tiles, epilogues follow
                # their last PV, so the in-order PE always has independent
                # work queued ahead of any instruction that waits on ACT/DVE
                # results. V rides inside g=0; Z(g+1) is produced at (g, 2).
                # With reps>1, the NEXT rep's chunk stats are doled out one
                # per few tiles (x is rep-invariant), removing them from the
                # serial head.
                pipelined_pre = pipe_stats and _rep + 1 < reps
                pending = (
                    stats_thunks()
                    + [lambda o=o: chain_pre(o) for o in range(NCH)]
                    if pipelined_pre
                    else []
                )
                T = NIG * NJP
                u8s = {}
                for t in range(T + lookahead):
                    if t < T:
                        g, jp = divmod(t, NJP)
                        if g == 0:
                            if jp == 0:
                                v_prod_pair(0)
                            if jp + 1 < NJP:
                                v_prod_pair(jp + 1)
                        if jp == 2 and g + 1 < NIG:
                            z_prod(g + 1)
                        if jp == 0:
                            acc[g] = psAcc.tile(
                                [P, NCH, 512], FP, tag="psa", name=f"acc{g}"
                            )
                            dps[g] = psD.tile([P, 512], FP, tag="psd", name=f"dps{g}")
                        u8s[t] = scores_exp(g, jp)
                        if pending and t >= 10 and (t - 10) % 2 == 0:
                            pending.pop(0)()
                    if t >= lookahead:
                        t2 = t - lookahead
                        g2, jp2 = divmod(t2, NJP)
                        pv_den(g2, jp2, u8s.pop(t2))
                        if jp2 == NJP - 1:
                            epilogue(g2)
                assert not pending, f"{len(pending)} stats thunks unplaced"

    if split:
        _split_waits(nc)
    return nc


BUILD = build_nc_fp8

_NC_CACHE = None


def _get_nc():
    global _NC_CACHE
    if _NC_CACHE is None:
        _NC_CACHE = build_nc_fp8()
    return _NC_CACHE


def make_in_maps(x, gn_scale, gn_bias, wq, bq, wk, bk, wv, bv, wp, bp):
    B = x.shape[0]
    f32 = np.float32

    def col2(v):  # [256] -> [128, 2], column o = channels o*128..o*128+127
        return np.ascontiguousarray(np.asarray(v, f32).reshape(NCH, P).T)

    wq = np.asarray(wq, f32)
    wk = np.asarray(wk, f32)
    M = wk.T @ wq                      # scores bilinear form: S = of^T M of
    mT = np.ascontiguousarray(M.T)
    w2cv = col2(wk.T @ np.asarray(bq, f32))
    wvT = np.ascontiguousarray(np.asarray(wv, f32).T)
    wpT = np.ascontiguousarray(np.asarray(wp, f32).T)
    bp2 = col2(np.asarray(wp, f32) @ np.asarray(bv, f32) + np.asarray(bp, f32))
    gns = col2(gn_scale)
    gnb = col2(gn_bias)
    gsel = np.kron(np.eye(P // GSIZE, dtype=f32), np.ones((GSIZE, GSIZE), f32))

    xr = np.asarray(x, f32).reshape(B, C, HW)
    in_maps = []
    for core in range(8):
        b, h = core // 2, core % 2
        # rotate so this core's query half sits at columns 0..HALF-1
        # (attention/groupnorm are token-order invariant)
        xfb = np.ascontiguousarray(np.roll(xr[b], -h * HALF, axis=1))
        in_maps.append(
            {
                "xf": xfb,
                "mT": mT, "wvT": wvT, "wpT": wpT,
                "w2c": w2cv, "bp2": bp2, "gns": gns, "gnb": gnb, "gsel": gsel,
            }
        )
    return in_maps


def assemble_out(results, B=4):
    out = np.empty((B, C, HW), np.float32)
    for core in range(8):
        b, h = core // 2, core % 2
        out[b, :, h * HALF : (h + 1) * HALF] = results[core]["y"]
    return out.reshape(B, C, 64, 64)


def kernel(**inputs):
    in_maps = make_in_maps(**inputs)
    try:
        nc = _get_nc()
        res = run_bass_kernel_spmd(nc, in_maps, list(range(8)))
    except Exception:
        # let a possibly-wedged device recover, then retry once
        import time as _time

        _time.sleep(90)
        nc = _get_nc()
        res = run_bass_kernel_spmd(nc, in_maps, list(range(8)))
    return assemble_out(res.results, B=inputs["x"].shape[0])


if __name__ == "__main__":
    rng = np.random.default_rng(0)
    ins = {
        "x": rng.standard_normal((4, C, 64, 64)).astype(np.float32),
        "gn_scale": np.ones(C, np.float32),
        "gn_bias": np.zeros(C, np.float32),
    }
    for w in ("wq", "wk", "wv", "wp"):
        ins[w] = (rng.standard_normal((C, C)) / 16.0).astype(np.float32)
    for b in ("bq", "bk", "bv", "bp"):
        ins[b] = np.zeros(C, np.float32)
    out = kernel(**ins)
    print(out.shape, out.dtype, float(np.abs(out).mean()))
